# revision 1
# baseline (speedup 1.0000x reference)
"""Diagonal-masked multi-head self-attention on 8 TRN2 NeuronCores.

Sharding: core c handles batch b = c // 2 and heads h0 = (c % 2) * 8 .. +8
(data parallel on B=4, tensor parallel over the 16 heads).  Each core
computes a partial output [S, D]; the host sums the two half-head partials
per batch and adds the output bias.

Per-core dataflow (bf16 matmuls, fp32 PSUM accumulation):
  - Host pre-transposes activations/weights so every matmul operand is
    already in its natural [K-on-partitions, free] layout.
  - Q/K projections produce QH^T / KH^T [dk, seq]; V produces VH [seq, dk].
    KH^T is stored twice with the other head's rows zeroed so score
    matmuls run with full K=128 weights (enables fast weight load).
  - Scores are computed transposed (S^T[t, q]); exp runs on the scalar
    engine straight out of PSUM (scale=1/sqrt(dk) folded in); the
    diagonal mask multiplies the one diagonal 128x128 block by (1 - I).
  - P^T @ V is computed as O^T with a ones column folded into the V
    weights, so each head's softmax denominator falls out of the same
    matmul (row 64 of each half's PSUM tile).
  - Normalization broadcasts the reciprocal denominator across
    partitions on GpSimd and multiplies during the PSUM->SBUF copy.
  - The output projection contracts O^T directly (it is already the
    lhsT the PE wants).
"""

import numpy as np
import ml_dtypes

B, S, D, H = 4, 2048, 1024, 16
DK = D // H
N_CORES = 8
HEADS_PER_CORE = H // 2


def build_attention_core(S=2048, DIN=1024, NH=8, DOUT=1024, aug_bias=False):
    """Build the per-core Tile program (strict phases, 1024-wide exp)."""
    import concourse.bacc as bacc
    import concourse.bass as bass
    import concourse.mybir as mybir
    import concourse.tile as tile

    fp32 = mybir.dt.float32
    bf16 = mybir.dt.bfloat16

    NP = NH // 2              # head pairs
    DC = NH * DK              # concat head dim on this core
    VW = 128                  # per-head V slot: [V(64) ones(1) pad(63)=1]
    NT = S // 128             # t tiles (key/value positions)
    NQ = S // 512             # q chunks of 512
    KA = DIN + 1 if aug_bias else DIN
    NK = (KA + 127) // 128    # contraction tiles for projections
    ND = (DOUT + 511) // 512  # output-dim chunks
    DCH = min(512, DOUT)

    assert S % 512 == 0 and DIN % 128 == 0 and DOUT % 512 in (0, DOUT)

    nc = bacc.Bacc(None, target_bir_lowering=False, debug=False)

    xq = nc.dram_tensor("xq", [KA, S], bf16, kind="ExternalInput")
    xk = nc.dram_tensor("xk", [KA, S], bf16, kind="ExternalInput")
    xv = nc.dram_tensor("xv", [KA, S], bf16, kind="ExternalInput")
    wq = nc.dram_tensor("wq", [KA, DC], bf16, kind="ExternalInput")
    wk = nc.dram_tensor("wk", [KA, DC], bf16, kind="ExternalInput")
    wv = nc.dram_tensor("wv", [KA, DC], bf16, kind="ExternalInput")
    wo = nc.dram_tensor("wo", [DC, DOUT], bf16, kind="ExternalInput")
    eyec = nc.dram_tensor("eyec", [128, 128], bf16, kind="ExternalInput")
    outp = nc.dram_tensor("outp", [S, DOUT], fp32, kind="ExternalOutput")

    def ksz(k):  # rows in contraction tile k
        return min(128, KA - k * 128)

    with tile.TileContext(nc) as tc:
        with (
            tc.tile_pool(name="persist", bufs=1) as persist,
            tc.tile_pool(name="xin", bufs=NK + 2) as xin,
            tc.tile_pool(name="win", bufs=1) as win,
            tc.tile_pool(name="epool", bufs=6) as epool,
            tc.tile_pool(name="npool", bufs=2) as npool,
            tc.tile_pool(name="opool", bufs=2) as opool,
            tc.tile_pool(name="scps", bufs=2, space="PSUM") as scps,
        ):
            # ---- persistent SBUF tensors -------------------------------
            qht = persist.tile([128, NP * S], bf16, tag="qht")       # pair-major
            khtp = persist.tile([128, 2 * NP * S], bf16, tag="khtp")  # zero-padded
            vh = persist.tile([128, NT * NH * VW], bf16, tag="vh")
            ot = persist.tile([128, NP * S], bf16, tag="ot")
            eye = persist.tile([128, 128], bf16, tag="eye")
            wo_sb = persist.tile([128, NP * DOUT], bf16, tag="wo")

            nc.sync.dma_start(eye[:], eyec[:])
            nc.vector.memset(vh[:], 1.0)
            nc.vector.memset(khtp[:], 0.0)

            # ---- phase A: projections ---------------------------------
            if True:
                for which, xdram, wdram in (("q", xq, wq), ("k", xk, wk)):
                    xt, wt = [], []
                    for k in range(NK):
                        xtile = xin.tile([128, S], bf16, tag="xt")
                        xt.append(xtile)
                        wtile = win.tile([128, DC], bf16, tag=f"w{which}{k}")
                        nc.sync.dma_start(wtile[: ksz(k), :], wdram[k * 128: k * 128 + ksz(k), :])
                        wt.append(wtile)
                    for n in range(NQ):
                        for k in range(NK):
                            nc.sync.dma_start(
                                xt[k][: ksz(k), n * 512:(n + 1) * 512],
                                xdram[k * 128: k * 128 + ksz(k), n * 512:(n + 1) * 512],
                            )
                    for n in range(NQ):
                        for m in range(NP):
                            ps = scps.tile([128, 512], fp32, tag="sc")
                            for k in range(NK):
                                nc.tensor.matmul(
                                    ps[:],
                                    wt[k][: ksz(k), m * 128:(m + 1) * 128],
                                    xt[k][: ksz(k), n * 512:(n + 1) * 512],
                                    start=(k == 0),
                                    stop=(k == NK - 1),
                                )
                            if which == "q":
                                nc.scalar.copy(
                                    qht[:, m * S + n * 512: m * S + (n + 1) * 512],
                                    ps[:],
                                )
                            else:
                                c0 = (2 * m) * S + n * 512
                                c1 = (2 * m + 1) * S + n * 512
                                nc.scalar.copy(
                                    khtp[0:64, c0: c0 + 512], ps[0:64, :]
                                )
                                nc.scalar.copy(
                                    khtp[64:128, c1: c1 + 512], ps[64:128, :]
                                )

                # V projection
                xt, wt = [], []
                for k in range(NK):
                    xtile = xin.tile([128, S], bf16, tag="xt")
                    xt.append(xtile)
                    wtile = win.tile([128, DC], bf16, tag=f"wv{k}")
                    nc.sync.dma_start(wtile[: ksz(k), :], wv[k * 128: k * 128 + ksz(k), :])
                    wt.append(wtile)
                for n in range(NQ):
                    for k in range(NK):
                        nc.sync.dma_start(
                            xt[k][: ksz(k), n * 512:(n + 1) * 512],
                            xv[k * 128: k * 128 + ksz(k), n * 512:(n + 1) * 512],
                        )
                for t in range(NT):
                    ps = scps.tile([128, DC], fp32, tag="sc")
                    for k in range(NK):
                        nc.tensor.matmul(
                            ps[:],
                            xt[k][: ksz(k), t * 128:(t + 1) * 128],
                            wt[k][: ksz(k), :],
                            start=(k == 0),
                            stop=(k == NK - 1),
                        )
                    base = t * NH * VW
                    nc.vector.tensor_copy(
                        vh[:, base: base + NH * VW].rearrange(
                            "p (h c) -> p h c", c=VW
                        )[:, :, 0:DK],
                        ps[:].rearrange("p (h c) -> p h c", c=DK),
                    )

            for p in range(NP):
                nc.sync.dma_start(
                    wo_sb[:, p * DOUT:(p + 1) * DOUT],
                    wo[p * 128:(p + 1) * 128, :],
                )

            # ---- phase B: attention (one 1024-wide exp per t) ---------
            scale = float(1.0 / np.sqrt(DK))
            with (
                tc.tile_pool(name="otaps", bufs=2, space="PSUM") as otaps,
                tc.tile_pool(name="otbps", bufs=2, space="PSUM") as otbps,
            ):
                for n in range(NQ):
                    for p in range(NP):
                        qof = p * S + n * 512
                        ota = otaps.tile([128, 512], fp32, tag="ota")
                        otb = otbps.tile([128, 512], fp32, tag="otb")
                        for t in range(NT):
                            sc = scps.tile([128, 1024], fp32, tag="sc")
                            nc.tensor.matmul(
                                sc[:, 0:512],
                                khtp[:, (2 * p) * S + t * 128: (2 * p) * S + (t + 1) * 128],
                                qht[:, qof: qof + 512],
                                start=True, stop=True,
                            )
                            nc.tensor.matmul(
                                sc[:, 512:1024],
                                khtp[:, (2 * p + 1) * S + t * 128: (2 * p + 1) * S + (t + 1) * 128],
                                qht[:, qof: qof + 512],
                                start=True, stop=True,
                            )
                            e = epool.tile([128, 1024], bf16, tag="e")
                            nc.scalar.activation(
                                e[:], sc[:], mybir.ActivationFunctionType.Exp,
                                scale=scale,
                            )
                            off = t * 128 - n * 512
                            if 0 <= off < 512:
                                nc.vector.tensor_mul(
                                    e[:, off: off + 128], e[:, off: off + 128], eye[:]
                                )
                                nc.vector.tensor_mul(
                                    e[:, 512 + off: 512 + off + 128],
                                    e[:, 512 + off: 512 + off + 128], eye[:]
                                )
                            vbase = t * NH * VW
                            nc.tensor.matmul(
                                ota[:],
                                vh[:, vbase + (2 * p) * VW: vbase + (2 * p + 1) * VW],
                                e[:, 0:512],
                                start=(t == 0), stop=(t == NT - 1),
                            )
                            nc.tensor.matmul(
                                otb[:],
                                vh[:, vbase + (2 * p + 1) * VW: vbase + (2 * p + 2) * VW],
                                e[:, 512:1024],
                                start=(t == 0), stop=(t == NT - 1),
                            )
                        # normalize (partition_broadcast reads physical p0)
                        rd = npool.tile([128, 1024], fp32, tag="rd")
                        nc.vector.reciprocal_approx_fast(rd[:, 0:512], ota[:])
                        nc.vector.reciprocal_approx_fast(rd[:, 512:1024], otb[:])
                        nc.sync.dma_start(rd[0:1, 0:512], rd[64:65, 0:512])
                        nc.sync.dma_start(rd[0:1, 512:1024], rd[64:65, 512:1024])
                        bca = npool.tile([64, 512], fp32, tag="bca")
                        bcb = npool.tile([64, 512], fp32, tag="bcb")
                        nc.gpsimd.partition_broadcast(bca[:], rd[0:1, 0:512], channels=64)
                        nc.gpsimd.partition_broadcast(bcb[:], rd[0:1, 512:1024], channels=64)
                        nc.vector.tensor_mul(
                            ot[0:64, qof: qof + 512], ota[0:64, :], bca[:]
                        )
                        tmpb = npool.tile([64, 512], bf16, tag="tmpb")
                        nc.vector.tensor_mul(tmpb[:], otb[0:64, :], bcb[:])
                        nc.sync.dma_start(ot[64:128, qof: qof + 512], tmpb[:])

            # ---- phase C: output projection ---------------------------
            if True:
                for qt in range(S // 128):
                    osb = opool.tile([128, DOUT], fp32, tag="osb")
                    for nd in range(ND):
                        ps = scps.tile([128, DCH], fp32, tag="sc")
                        for p in range(NP):
                            nc.tensor.matmul(
                                ps[:],
                                ot[:, p * S + qt * 128: p * S + (qt + 1) * 128],
                                wo_sb[:, p * DOUT + nd * DCH: p * DOUT + nd * DCH + DCH],
                                start=(p == 0), stop=(p == NP - 1),
                            )
                        nc.vector.tensor_copy(osb[:, nd * DCH:(nd + 1) * DCH], ps[:])
                    nc.sync.dma_start(outp[qt * 128:(qt + 1) * 128, :], osb[:])

    nc.compile()
    return nc


def _bf16(a):
    return np.ascontiguousarray(a).astype(ml_dtypes.bfloat16)


def _prep_core_inputs(q, k, v, Wq, bq, Wk, bk, Wv, bv, Wo, aug_bias):
    """Per-core host-side slicing/transposition. Returns list of 8 dicts."""
    eyec = _bf16(1.0 - np.eye(128, dtype=np.float32))
    maps = []
    for c in range(N_CORES):
        b = c // 2
        h0 = (c % 2) * HEADS_PER_CORE
        r0, r1 = h0 * DK, (h0 + HEADS_PER_CORE) * DK
        m = {}
        for name, x in (("xq", q[b]), ("xk", k[b]), ("xv", v[b])):
            xt = x.T  # [D, S]
            if aug_bias:
                xt = np.concatenate([xt, np.ones((1, S), np.float32)], axis=0)
            m[name] = _bf16(xt)
        for name, W, bias in (("wq", Wq, bq), ("wk", Wk, bk), ("wv", Wv, bv)):
            wt = W[r0:r1, :].T  # [D, DC]
            if aug_bias:
                wt = np.concatenate([wt, bias[None, r0:r1]], axis=0)
            m[name] = _bf16(wt)
        m["wo"] = _bf16(Wo[:, r0:r1].T)  # [DC, D]
        m["eyec"] = eyec
        maps.append(m)
    return maps


_PROGRAM_CACHE = {}


def _get_program(aug_bias):
    if aug_bias not in _PROGRAM_CACHE:
        _PROGRAM_CACHE[aug_bias] = build_attention_core(
            S=S, DIN=D, NH=HEADS_PER_CORE, DOUT=D, aug_bias=aug_bias
        )
    return _PROGRAM_CACHE[aug_bias]


def _reference_fallback(q, k, v, Wq, bq, Wk, bk, Wv, bv, Wo, bo, mask):
    """Pure-numpy fallback for unexpected mask patterns."""
    out = np.empty((B, S, D), np.float32)
    msk = np.broadcast_to(mask.reshape(mask.shape[-2], mask.shape[-1]), (S, S))
    for b in range(B):
        qh = (q[b] @ Wq.T + bq).reshape(S, H, DK).transpose(1, 0, 2)
        kh = (k[b] @ Wk.T + bk).reshape(S, H, DK).transpose(1, 0, 2)
        vh = (v[b] @ Wv.T + bv).reshape(S, H, DK).transpose(1, 0, 2)
        acc = np.empty((H, S, DK), np.float32)
        for h in range(H):
            s = (qh[h] @ kh[h].T) / np.float32(np.sqrt(DK))
            s = np.where(msk == 0, np.finfo(np.float32).min, s)
            s = s - s.max(axis=-1, keepdims=True)
            e = np.exp(s)
            p = e / e.sum(axis=-1, keepdims=True)
            acc[h] = p @ vh[h]
        o = acc.transpose(1, 0, 2).reshape(S, D)
        out[b] = o @ Wo.T + bo
    return out


def kernel(q, k, v, Wq, bq, Wk, bk, Wv, bv, Wo, bo, mask, _trace=False):
    from concourse.bass_utils import run_bass_kernel_spmd

    q = np.asarray(q, np.float32)
    k = np.asarray(k, np.float32)
    v = np.asarray(v, np.float32)
    Wq, bq = np.asarray(Wq, np.float32), np.asarray(bq, np.float32)
    Wk, bk = np.asarray(Wk, np.float32), np.asarray(bk, np.float32)
    Wv, bv = np.asarray(Wv, np.float32), np.asarray(bv, np.float32)
    Wo, bo = np.asarray(Wo, np.float32), np.asarray(bo, np.float32)
    mask = np.asarray(mask)

    expected_mask = 1 - np.eye(S, dtype=np.int32)
    if not np.array_equal(mask.reshape(-1, S, S)[0].astype(np.int32), expected_mask):
        return _reference_fallback(q, k, v, Wq, bq, Wk, bk, Wv, bv, Wo, bo, mask)

    aug_bias = bool(np.any(bq) or np.any(bk) or np.any(bv))
    nc = _get_program(aug_bias)
    in_maps = _prep_core_inputs(q, k, v, Wq, bq, Wk, bk, Wv, bv, Wo, aug_bias)
    res = run_bass_kernel_spmd(
        nc, in_maps, core_ids=list(range(N_CORES)), trace=_trace
    )
    out = np.empty((B, S, D), np.float32)
    for b in range(B):
        out[b] = res.results[2 * b]["outp"] + res.results[2 * b + 1]["outp"] + bo
    if _trace:
        kernel.last_results = res
    return out



# revision 5
# speedup vs baseline: 1.0046x; 1.0046x over previous
"""Diagonal-masked multi-head self-attention on 8 TRN2 NeuronCores.

Sharding: core c handles batch b = c // 2 and heads h0 = (c % 2) * 8 .. +8
(data parallel on B=4, tensor parallel over the 16 heads).  Each core
computes a partial output [S, D]; the host sums the two half-head partials
per batch and adds the output bias.

Per-core design (v2, software-pipelined):
  The softmax exp on the Scalar engine (256 tiles x [128,1024] @ ~1.1us) is
  the hard floor (~290us), so the whole kernel is paced by the exp stream:

  - Attention runs as one flat stream of 256 iterations (pair-major, then
    q-chunk, then t-block).  Each iteration emits the two score matmuls
    (row-tiled: the K=64 head halves run concurrently on PE array tiles
    (0,0)/(64,0)), the exp, the (rare) diagonal-mask multiply, and the PV
    matmuls of the iteration LAG=3 behind (so exp latency never stalls the
    PE).
  - All projection work (Q/K per pair, V, and the output projection) is
    chopped into ~8-matmul chains and drip-fed into the same instruction
    stream as credit-paced "background" PE work, so phases fully overlap:
    the PE projects pair p+1 while the Scalar engine exps pair p.
  - x inputs are streamed from DRAM in per-chunk tile sets (double
    buffered, DMA emitted one chain ahead); V is projected in two column
    passes (pair 0 at N=128 so attention starts early, pairs 1-3 at N=384
    in background).
  - PSUM (8 banks): scores 2x[128,1024], accumulator 1x[128,1024]
    (evicted fp32 to SBUF right after each 16-t accumulation), projection
    chains 2x[128,512].
  - Scalar does ONLY exp.  All PSUM evictions are Vector copies.  The
    softmax denominator falls out of a ones column in the V weights
    (row 64 of the accumulator); reciprocal+broadcast+multiply run on
    Vector/GpSimd off the critical path.
"""

import numpy as np
import ml_dtypes

B, S, D, H = 4, 2048, 1024, 16
DK = D // H
N_CORES = 8
HEADS_PER_CORE = H // 2


def build_attention_core(S=2048, DIN=1024, NH=8, DOUT=1024, aug_bias=False):
    from collections import deque

    import concourse.bacc as bacc
    import concourse.mybir as mybir
    import concourse.tile as tile

    fp32 = mybir.dt.float32
    bf16 = mybir.dt.bfloat16

    NP = NH // 2              # head pairs per core (4)
    DC = NH * DK              # concat head dim on this core (512)
    NT = S // 128             # t tiles (16)
    NQ = S // 512             # q chunks (4)
    KA = DIN + 1 if aug_bias else DIN
    NK = (KA + 127) // 128    # contraction tiles for projections
    VW = 66                   # per-head V slot: V(64) + ones(1) + pad(1)
    LAG = 3                   # pv trails sc/exp by LAG iterations
    ND = DOUT // 512

    nc = bacc.Bacc(None, target_bir_lowering=False, debug=False)

    xq = nc.dram_tensor("xq", [KA, S], bf16, kind="ExternalInput")
    xk = nc.dram_tensor("xk", [KA, S], bf16, kind="ExternalInput")
    xv = nc.dram_tensor("xv", [KA, S], bf16, kind="ExternalInput")
    wq = nc.dram_tensor("wq", [KA, DC], bf16, kind="ExternalInput")
    wk = nc.dram_tensor("wk", [KA, DC], bf16, kind="ExternalInput")
    wv = nc.dram_tensor("wv", [KA, DC], bf16, kind="ExternalInput")
    wo = nc.dram_tensor("wo", [DC, DOUT], bf16, kind="ExternalInput")
    eyec = nc.dram_tensor("eyec", [128, 128], bf16, kind="ExternalInput")
    outp = nc.dram_tensor("outp", [S, DOUT], fp32, kind="ExternalOutput")

    def ksz(k):
        return min(128, KA - k * 128)

    scale = float(1.0 / np.sqrt(DK))

    with tile.TileContext(nc) as tc:
        with (
            tc.tile_pool(name="persist", bufs=1) as persist,
            tc.tile_pool(name="xqp", bufs=2) as xqp,
            tc.tile_pool(name="xkp", bufs=2) as xkp,
            tc.tile_pool(name="xvp", bufs=2) as xvp,
            tc.tile_pool(name="epool", bufs=LAG + 5) as epool,
            tc.tile_pool(name="opool", bufs=2) as opool,
            tc.tile_pool(name="bpool", bufs=1) as bpool,
            tc.tile_pool(name="cpool", bufs=2) as cpool,
            tc.tile_pool(name="scps", bufs=2, space="PSUM") as scps,
            tc.tile_pool(name="acps", bufs=1, space="PSUM") as acps,
            tc.tile_pool(name="ppps", bufs=2, space="PSUM") as ppps,
        ):
            # ---- persistent SBUF ------------------------------------------
            qht = [persist.tile([128, S], bf16, tag=f"qht{p}", name=f"qht{p}") for p in range(NP)]
            kht = [persist.tile([128, S], bf16, tag=f"kht{p}", name=f"kht{p}") for p in range(NP)]
            vh = [persist.tile([128, NT * 2 * VW], bf16, tag=f"vh{p}", name=f"vh{p}") for p in range(NP)]
            ot = [persist.tile([128, S], bf16, tag=f"ot{p}", name=f"ot{p}") for p in range(NP)]
            eye = persist.tile([128, 128], bf16, tag="eye")
            wo_sb = persist.tile([128, NP * DOUT], bf16, tag="wo")
            wtq = [persist.tile([128, DC], bf16, tag=f"wtq{k}", name=f"wtq{k}") for k in range(NK)]
            wtk = [persist.tile([128, DC], bf16, tag=f"wtk{k}", name=f"wtk{k}") for k in range(NK)]
            wtv = [persist.tile([128, DC], bf16, tag=f"wtv{k}", name=f"wtv{k}") for k in range(NK)]

            # ---- helpers ---------------------------------------------------
            def dma_x_chunk(pool, xdram, tag, n):
                tiles = []
                for k in range(NK):
                    t_ = pool.tile([128, 512], bf16, tag=f"{tag}{k}", name=f"{tag}{k}")
                    nc.sync.dma_start(
                        t_[: ksz(k), :],
                        xdram[k * 128: k * 128 + ksz(k), n * 512:(n + 1) * 512],
                    )
                    tiles.append(t_)
                return tiles

            def chain_q(p, n, xt):
                ps = ppps.tile([128, 512], fp32, tag="pp")
                for k in range(NK):
                    nc.tensor.matmul(
                        ps[:], wtq[k][: ksz(k), p * 128:(p + 1) * 128],
                        xt[k][: ksz(k), :],
                        start=(k == 0), stop=(k == NK - 1),
                    )
                nc.vector.tensor_copy(qht[p][:, n * 512:(n + 1) * 512], ps[:])

            def chain_k(p, c, xt):
                ps = ppps.tile([128, 512], fp32, tag="pp")
                for k in range(NK):
                    nc.tensor.matmul(
                        ps[:], wtk[k][: ksz(k), p * 128:(p + 1) * 128],
                        xt[k][: ksz(k), :],
                        start=(k == 0), stop=(k == NK - 1),
                    )
                nc.vector.tensor_copy(kht[p][:, c * 512:(c + 1) * 512], ps[:])

            def chain_v(t, xt, p0, p1):
                """V projection for t-block t, pairs [p0, p1) -> vh slots."""
                npair = p1 - p0
                col = t * 128 % 512
                ps = ppps.tile([128, 512], fp32, tag="pp")
                for k in range(NK):
                    nc.tensor.matmul(
                        ps[:, : npair * 128],
                        xt[k][: ksz(k), col: col + 128],
                        wtv[k][: ksz(k), p0 * 128: p1 * 128],
                        start=(k == 0), stop=(k == NK - 1),
                    )
                for p in range(p0, p1):
                    dst = vh[p][:, t * 2 * VW: t * 2 * VW + 2 * VW].rearrange(
                        "p (h c) -> p h c", c=VW
                    )[:, :, 0:DK]
                    src = ps[:, (p - p0) * 128: (p - p0 + 1) * 128].rearrange(
                        "p (h c) -> p h c", c=DK
                    )
                    nc.vector.tensor_copy(dst, src)

            c_osb = {}

            def chain_c(qt, nd):
                """Output projection for row block qt, 512-col half nd."""
                ps = ppps.tile([128, 512], fp32, tag="pp")
                for p in range(NP):
                    nc.tensor.matmul(
                        ps[:], ot[p][:, qt * 128:(qt + 1) * 128],
                        wo_sb[:, p * DOUT + nd * 512: p * DOUT + (nd + 1) * 512],
                        start=(p == 0), stop=(p == NP - 1),
                    )
                if nd == 0:
                    c_osb[qt] = cpool.tile([128, DOUT], fp32, tag="osb", name="osb")
                osb = c_osb[qt]
                nc.vector.tensor_copy(osb[:, nd * 512:(nd + 1) * 512], ps[:])
                if nd == ND - 1:
                    del c_osb[qt]
                    nc.sync.dma_start(outp[qt * 128:(qt + 1) * 128, :], osb[:])

            def normalize(p, n, acc):
                # evict numerator+denominator fp32 (frees the PSUM accumulator)
                osb = opool.tile([128, 1024], fp32, tag="nosb")
                nc.vector.tensor_copy(osb[0:65, :], acc[0:65, :])
                den = bpool.tile([128, 1024], fp32, tag="den")
                nc.sync.dma_start(den[0:1, :], osb[64:65, :])
                rd = bpool.tile([128, 1024], fp32, tag="rd")
                nc.vector.reciprocal_approx_fast(rd[0:1, :], den[0:1, :])
                bca = bpool.tile([64, 1024], fp32, tag="bca")
                nc.gpsimd.partition_broadcast(bca[:], rd[0:1, :], channels=64)
                qof = n * 512
                nc.vector.tensor_mul(
                    ot[p][0:64, qof: qof + 512], osb[0:64, 0:512], bca[:, 0:512]
                )
                tmpb = bpool.tile([64, 512], bf16, tag="tmpb")
                nc.vector.tensor_mul(tmpb[:], osb[0:64, 512:1024], bca[:, 512:1024])
                nc.sync.dma_start(ot[p][64:128, qof: qof + 512], tmpb[:])

            # ---- background queue (credit-paced PE work) -------------------
            bg = deque()
            bg_pair = [deque() for _ in range(NP)]
            credit = [0.0]

            def run_bg(rate, cap=4500.0):
                credit[0] = min(credit[0] + rate, cap)
                while bg and (credit[0] > 0 or bg[0][0] == 0):
                    cost, thunk = bg.popleft()
                    thunk()
                    credit[0] -= cost

            def refill(p):
                src = bg_pair[p]
                while src and len(bg) < 8:
                    bg.append(src.popleft())

            def drain_all():
                for p in range(NP):
                    while bg_pair[p]:
                        bg.append(bg_pair[p].popleft())
                while bg:
                    _, thunk = bg.popleft()
                    thunk()

            store = {}

            def mk_dma(pool, xdram, tag, n, key):
                def f():
                    store[key] = dma_x_chunk(pool, xdram, tag, n)
                return (0, f)

            def mk_q(p, n):
                return (NK * 512, lambda: chain_q(p, n, store[("q", p, n)]))

            def mk_k(p, c):
                return (NK * 512, lambda: chain_k(p, c, store[("k", p, c)]))

            def mk_v(t, key, p0, p1):
                return (NK * 128 * (p1 - p0), lambda: chain_v(t, store[key], p0, p1))

            # ---- prologue --------------------------------------------------
            # weights needed early; wo (output proj) deferred to background
            for k in range(NK):
                nc.sync.dma_start(wtk[k][: ksz(k), :], wk[k * 128: k * 128 + ksz(k), :])
            for k in range(NK):
                nc.sync.dma_start(wtq[k][: ksz(k), :], wq[k * 128: k * 128 + ksz(k), :])
            for k in range(NK):
                nc.sync.dma_start(wtv[k][: ksz(k), :], wv[k * 128: k * 128 + ksz(k), :])
            nc.sync.dma_start(eye[:], eyec[:])
            for p in range(NP):
                nc.vector.memset(vh[p][:], 1.0)

            store[("k", 0, 0)] = dma_x_chunk(xkp, xk, "xk", 0)
            store[("q", 0, 0)] = dma_x_chunk(xqp, xq, "xq", 0)
            store[("k", 0, 1)] = dma_x_chunk(xkp, xk, "xk", 1)
            chain_k(0, 0, store[("k", 0, 0)])
            store[("v", 0)] = dma_x_chunk(xvp, xv, "xv", 0)
            chain_q(0, 0, store[("q", 0, 0)])
            store[("q", 0, 1)] = dma_x_chunk(xqp, xq, "xq", 1)
            chain_k(0, 1, store[("k", 0, 1)])
            store[("k", 0, 2)] = dma_x_chunk(xkp, xk, "xk", 2)
            chain_k(0, 2, store[("k", 0, 2)])
            store[("k", 0, 3)] = dma_x_chunk(xkp, xk, "xk", 3)
            chain_k(0, 3, store[("k", 0, 3)])
            chain_q(0, 1, store[("q", 0, 1)])
            for t in range(4):
                chain_v(t, store[("v", 0)], 0, 1)

            # ---- background schedules per pair -----------------------------
            # pair 0: V(p0) t4..15 (tight deadlines: pv needs V(t) at iter
            # t+LAG), then Q(0,n2..3), then K(1)/Q(1), then V(pairs1-3) t0..7.
            b0 = bg_pair[0]
            b0.append(mk_dma(xvp, xv, "xv", 1, ("v", 1)))
            b0.append(mk_dma(xvp, xv, "xv", 2, ("v", 2)))
            for t in range(4, 8):
                b0.append(mk_v(t, ("v", 1), 0, 1))
            b0.append(mk_dma(xvp, xv, "xv", 3, ("v", 3)))
            for t in range(8, 12):
                b0.append(mk_v(t, ("v", 2), 0, 1))
            for t in range(12, 16):
                b0.append(mk_v(t, ("v", 3), 0, 1))
            b0.append(mk_dma(xqp, xq, "xq", 2, ("q", 0, 2)))
            b0.append(mk_q(0, 2))
            b0.append(mk_dma(xqp, xq, "xq", 3, ("q", 0, 3)))
            b0.append(mk_q(0, 3))
            b0.append(mk_dma(xkp, xk, "xk", 0, ("k", 1, 0)))
            b0.append(mk_dma(xkp, xk, "xk", 1, ("k", 1, 1)))
            b0.append(mk_k(1, 0))
            b0.append(mk_dma(xkp, xk, "xk", 2, ("k", 1, 2)))
            b0.append(mk_k(1, 1))
            b0.append(mk_dma(xkp, xk, "xk", 3, ("k", 1, 3)))
            b0.append(mk_k(1, 2))
            b0.append(mk_dma(xqp, xq, "xq", 0, ("q", 1, 0)))
            b0.append(mk_k(1, 3))
            b0.append(mk_dma(xqp, xq, "xq", 1, ("q", 1, 1)))
            b0.append(mk_q(1, 0))
            b0.append(mk_dma(xqp, xq, "xq", 2, ("q", 1, 2)))
            b0.append(mk_q(1, 1))
            b0.append(mk_dma(xqp, xq, "xq", 3, ("q", 1, 3)))
            b0.append(mk_q(1, 2))
            b0.append(mk_dma(xvp, xv, "xv", 0, ("vb", 0)))
            b0.append(mk_q(1, 3))
            b0.append(mk_dma(xvp, xv, "xv", 1, ("vb", 1)))
            for t in range(0, 4):
                b0.append(mk_v(t, ("vb", 0), 1, 4))
            for t in range(4, 8):
                b0.append(mk_v(t, ("vb", 1), 1, 4))

            # pair 1: V(pairs1-3) t8..15 (deadlines early in pair 1... these
            # are for pv of pair 1 itself at iters 11..18), wo DMA, K(2)/Q(2).
            b1 = bg_pair[1]
            b1.append(mk_dma(xvp, xv, "xv", 2, ("vb", 2)))
            b1.append(mk_dma(xvp, xv, "xv", 3, ("vb", 3)))
            for t in range(8, 12):
                b1.append(mk_v(t, ("vb", 2), 1, 4))
            for t in range(12, 16):
                b1.append(mk_v(t, ("vb", 3), 1, 4))

            def dma_wo():
                for p in range(NP):
                    nc.sync.dma_start(
                        wo_sb[:, p * DOUT:(p + 1) * DOUT], wo[p * 128:(p + 1) * 128, :]
                    )
            b1.append((0, dma_wo))
            for c in range(4):
                b1.append(mk_dma(xkp, xk, "xk", c, ("k", 2, c)))
                b1.append(mk_k(2, c))
            for n in range(4):
                b1.append(mk_dma(xqp, xq, "xq", n, ("q", 2, n)))
                b1.append(mk_q(2, n))

            # pair 2: K(3)/Q(3)
            b2 = bg_pair[2]
            for c in range(4):
                b2.append(mk_dma(xkp, xk, "xk", c, ("k", 3, c)))
                b2.append(mk_k(3, c))
            for n in range(4):
                b2.append(mk_dma(xqp, xq, "xq", n, ("q", 3, n)))
                b2.append(mk_q(3, n))

            # pair 3: output-projection chains are queued by normalize()

            # ---- main attention stream -------------------------------------
            iters = [(p, n, t) for p in range(NP) for n in range(NQ) for t in range(NT)]
            NIT = len(iters)
            e_buf = {}
            acc_buf = {}

            def emit_sc_exp(i):
                p, n, t = iters[i]
                sc = scps.tile([128, 1024], fp32, tag="sc")
                # row-tiled score matmuls: head A on PE tile (0,0), B on (64,0)
                nc.tensor.matmul(
                    sc[:, 0:512],
                    kht[p][0:64, t * 128:(t + 1) * 128],
                    qht[p][0:64, n * 512:(n + 1) * 512],
                    start=True, stop=True,
                )
                nc.tensor.matmul(
                    sc[:, 512:1024],
                    kht[p][64:128, t * 128:(t + 1) * 128],
                    qht[p][64:128, n * 512:(n + 1) * 512],
                    start=True, stop=True,
                )
                e = epool.tile([128, 1024], bf16, tag="e")
                nc.scalar.activation(
                    e[:], sc[:], mybir.ActivationFunctionType.Exp, scale=scale
                )
                off = t * 128 - n * 512
                if 0 <= off < 512:
                    nc.vector.tensor_mul(e[:, off: off + 128], e[:, off: off + 128], eye[:])
                    nc.vector.tensor_mul(
                        e[:, 512 + off: 512 + off + 128],
                        e[:, 512 + off: 512 + off + 128], eye[:],
                    )
                e_buf[i] = e

            def emit_pv(j):
                p, n, t = iters[j]
                if t == 0:
                    acc_buf[(p, n)] = acps.tile([128, 1024], fp32, tag="acc", name="acc")
                acc = acc_buf[(p, n)]
                e = e_buf.pop(j)
                vbase = t * 2 * VW
                nc.tensor.matmul(
                    acc[0:65, 0:512],
                    vh[p][:, vbase: vbase + 65],
                    e[:, 0:512],
                    start=(t == 0), stop=(t == NT - 1),
                )
                nc.tensor.matmul(
                    acc[0:65, 512:1024],
                    vh[p][:, vbase + VW: vbase + VW + 65],
                    e[:, 512:1024],
                    start=(t == 0), stop=(t == NT - 1),
                )
                if t == NT - 1:
                    normalize(p, n, acc_buf.pop((p, n)))
                    if p == NP - 1:
                        for qt in range(n * 4, n * 4 + 4):
                            for nd in range(ND):
                                bg_pair[3].append(
                                    (NP * 512, (lambda qt_=qt, nd_=nd: chain_c(qt_, nd_)))
                                )

            for i in range(NIT):
                p, n, t = iters[i]
                emit_sc_exp(i)
                if i >= LAG:
                    emit_pv(i - LAG)
                refill(p)
                # bg credit per iteration: generous for pairs 0-1 (V-proj
                # deadlines), nominal afterwards.
                run_bg(1400.0 if p <= 1 else 1150.0)
            for j in range(NIT - LAG, NIT):
                emit_pv(j)
                run_bg(1400.0)
            drain_all()

    nc.compile()
    return nc


def _bf16(a):
    return np.ascontiguousarray(a).astype(ml_dtypes.bfloat16)


def _prep_core_inputs(q, k, v, Wq, bq, Wk, bk, Wv, bv, Wo, aug_bias):
    """Per-core host-side slicing/transposition. Returns list of 8 dicts."""
    eyec = _bf16(1.0 - np.eye(128, dtype=np.float32))
    maps = []
    for c in range(N_CORES):
        b = c // 2
        h0 = (c % 2) * HEADS_PER_CORE
        r0, r1 = h0 * DK, (h0 + HEADS_PER_CORE) * DK
        m = {}
        for name, x in (("xq", q[b]), ("xk", k[b]), ("xv", v[b])):
            xt = x.T  # [D, S]
            if aug_bias:
                xt = np.concatenate([xt, np.ones((1, S), np.float32)], axis=0)
            m[name] = _bf16(xt)
        for name, W, bias in (("wq", Wq, bq), ("wk", Wk, bk), ("wv", Wv, bv)):
            wt = W[r0:r1, :].T  # [D, DC]
            if aug_bias:
                wt = np.concatenate([wt, bias[None, r0:r1]], axis=0)
            m[name] = _bf16(wt)
        m["wo"] = _bf16(Wo[:, r0:r1].T)  # [DC, D]
        m["eyec"] = eyec
        maps.append(m)
    return maps


_PROGRAM_CACHE = {}


def _get_program(aug_bias):
    if aug_bias not in _PROGRAM_CACHE:
        _PROGRAM_CACHE[aug_bias] = build_attention_core(
            S=S, DIN=D, NH=HEADS_PER_CORE, DOUT=D, aug_bias=aug_bias
        )
    return _PROGRAM_CACHE[aug_bias]


def _reference_fallback(q, k, v, Wq, bq, Wk, bk, Wv, bv, Wo, bo, mask):
    """Pure-numpy fallback for unexpected mask patterns."""
    out = np.empty((B, S, D), np.float32)
    msk = np.broadcast_to(mask.reshape(mask.shape[-2], mask.shape[-1]), (S, S))
    for b in range(B):
        qh = (q[b] @ Wq.T + bq).reshape(S, H, DK).transpose(1, 0, 2)
        kh = (k[b] @ Wk.T + bk).reshape(S, H, DK).transpose(1, 0, 2)
        vh = (v[b] @ Wv.T + bv).reshape(S, H, DK).transpose(1, 0, 2)
        acc = np.empty((H, S, DK), np.float32)
        for h in range(H):
            s = (qh[h] @ kh[h].T) / np.float32(np.sqrt(DK))
            s = np.where(msk == 0, np.finfo(np.float32).min, s)
            s = s - s.max(axis=-1, keepdims=True)
            e = np.exp(s)
            p = e / e.sum(axis=-1, keepdims=True)
            acc[h] = p @ vh[h]
        o = acc.transpose(1, 0, 2).reshape(S, D)
        out[b] = o @ Wo.T + bo
    return out


def kernel(q, k, v, Wq, bq, Wk, bk, Wv, bv, Wo, bo, mask, _trace=False):
    from concourse.bass_utils import run_bass_kernel_spmd

    q = np.asarray(q, np.float32)
    k = np.asarray(k, np.float32)
    v = np.asarray(v, np.float32)
    Wq, bq = np.asarray(Wq, np.float32), np.asarray(bq, np.float32)
    Wk, bk = np.asarray(Wk, np.float32), np.asarray(bk, np.float32)
    Wv, bv = np.asarray(Wv, np.float32), np.asarray(bv, np.float32)
    Wo, bo = np.asarray(Wo, np.float32), np.asarray(bo, np.float32)
    mask = np.asarray(mask)

    expected_mask = 1 - np.eye(S, dtype=np.int32)
    if not np.array_equal(mask.reshape(-1, S, S)[0].astype(np.int32), expected_mask):
        return _reference_fallback(q, k, v, Wq, bq, Wk, bk, Wv, bv, Wo, bo, mask)

    aug_bias = bool(np.any(bq) or np.any(bk) or np.any(bv))
    nc = _get_program(aug_bias)
    in_maps = _prep_core_inputs(q, k, v, Wq, bq, Wk, bk, Wv, bv, Wo, aug_bias)
    res = run_bass_kernel_spmd(
        nc, in_maps, core_ids=list(range(N_CORES)), trace=_trace
    )
    out = np.empty((B, S, D), np.float32)
    for b in range(B):
        out[b] = res.results[2 * b]["outp"] + res.results[2 * b + 1]["outp"] + bo
    if _trace:
        kernel.last_results = res
    return out


# revision 8
# speedup vs baseline: 1.1117x; 1.1065x over previous
"""Diagonal-masked multi-head self-attention on 8 TRN2 NeuronCores.

Sharding: core c handles batch b = c // 2 and heads h0 = (c % 2) * 8 .. +8
(data parallel on B=4, tensor parallel over the 16 heads).  Each core
computes a partial output [S, D]; the host sums the two half-head partials
per batch and adds the output bias.

Per-core design (v2, software-pipelined):
  The softmax exp on the Scalar engine (256 tiles x [128,1024] @ ~1.1us) is
  the hard floor (~290us), so the whole kernel is paced by the exp stream:

  - Attention runs as one flat stream of 256 iterations (pair-major, then
    q-chunk, then t-block).  Each iteration emits the two score matmuls
    (row-tiled: the K=64 head halves run concurrently on PE array tiles
    (0,0)/(64,0)), the exp, the (rare) diagonal-mask multiply, and the PV
    matmuls of the iteration LAG=3 behind (so exp latency never stalls the
    PE).
  - All projection work (Q/K per pair, V, and the output projection) is
    chopped into ~8-matmul chains and drip-fed into the same instruction
    stream as credit-paced "background" PE work, so phases fully overlap:
    the PE projects pair p+1 while the Scalar engine exps pair p.
  - x inputs are streamed from DRAM in per-chunk tile sets (double
    buffered, DMA emitted one chain ahead); V is projected in two column
    passes (pair 0 at N=128 so attention starts early, pairs 1-3 at N=384
    in background).
  - PSUM (8 banks): scores 2x[128,1024], accumulator 1x[128,1024]
    (evicted fp32 to SBUF right after each 16-t accumulation), projection
    chains 2x[128,512].
  - Scalar does ONLY exp.  All PSUM evictions are Vector copies.  The
    softmax denominator falls out of a ones column in the V weights
    (row 64 of the accumulator); reciprocal+broadcast+multiply run on
    Vector/GpSimd off the critical path.
"""

import numpy as np
import ml_dtypes

B, S, D, H = 4, 2048, 1024, 16
DK = D // H
N_CORES = 8
HEADS_PER_CORE = H // 2


def build_attention_core(S=2048, DIN=1024, NH=8, DOUT=1024, aug_bias=False):
    from collections import deque

    import concourse.bacc as bacc
    import concourse.mybir as mybir
    import concourse.tile as tile

    fp32 = mybir.dt.float32
    bf16 = mybir.dt.bfloat16

    NP = NH // 2              # head pairs per core (4)
    DC = NH * DK              # concat head dim on this core (512)
    NT = S // 128             # t tiles (16)
    NQ = S // 512             # q chunks (4)
    KA = DIN + 1 if aug_bias else DIN
    NK = (KA + 127) // 128    # contraction tiles for projections
    VW = 66                   # per-head V slot: V(64) + ones(1) + pad(1)
    LAG = 3                   # pv trails sc/exp by LAG iterations
    ND = DOUT // 512

    nc = bacc.Bacc(None, target_bir_lowering=False, debug=False)

    xq = nc.dram_tensor("xq", [KA, S], bf16, kind="ExternalInput")
    xk = nc.dram_tensor("xk", [KA, S], bf16, kind="ExternalInput")
    xv = nc.dram_tensor("xv", [KA, S], bf16, kind="ExternalInput")
    wq = nc.dram_tensor("wq", [KA, DC], bf16, kind="ExternalInput")
    wk = nc.dram_tensor("wk", [KA, DC], bf16, kind="ExternalInput")
    wv = nc.dram_tensor("wv", [KA, DC], bf16, kind="ExternalInput")
    wo = nc.dram_tensor("wo", [DC, DOUT], bf16, kind="ExternalInput")
    eyec = nc.dram_tensor("eyec", [128, 128], bf16, kind="ExternalInput")
    outp = nc.dram_tensor("outp", [S, DOUT], fp32, kind="ExternalOutput")

    def ksz(k):
        return min(128, KA - k * 128)

    scale = float(1.0 / np.sqrt(DK))

    with tile.TileContext(nc) as tc:
        with (
            tc.tile_pool(name="persist", bufs=1) as persist,
            tc.tile_pool(name="xqp", bufs=2) as xqp,
            tc.tile_pool(name="xvp", bufs=2) as xvp,
            tc.tile_pool(name="epool", bufs=LAG + 4) as epool,
            tc.tile_pool(name="opool", bufs=2) as opool,
            tc.tile_pool(name="bpool", bufs=1) as bpool,
            tc.tile_pool(name="cpool", bufs=2) as cpool,
            tc.tile_pool(name="scps", bufs=2, space="PSUM") as scps,
            tc.tile_pool(name="acps", bufs=1, space="PSUM") as acps,
            tc.tile_pool(name="ppps", bufs=2, space="PSUM") as ppps,
        ):
            # ---- persistent SBUF ------------------------------------------
            qht = [persist.tile([128, S], bf16, tag=f"qht{p}", name=f"qht{p}") for p in range(NP)]
            kht = [persist.tile([128, S], bf16, tag=f"kht{p}", name=f"kht{p}") for p in range(NP)]
            vh = [persist.tile([128, NT * 2 * VW], bf16, tag=f"vh{p}", name=f"vh{p}") for p in range(NP)]
            ot = [persist.tile([128, S], bf16, tag=f"ot{p}", name=f"ot{p}") for p in range(NP)]
            eye = persist.tile([128, 128], bf16, tag="eye")
            wo_sb = persist.tile([128, NP * DOUT], bf16, tag="wo")
            wtq = [persist.tile([128, DC], bf16, tag=f"wtq{k}", name=f"wtq{k}") for k in range(NK)]
            wtk = [persist.tile([128, DC], bf16, tag=f"wtk{k}", name=f"wtk{k}") for k in range(NK)]
            wtv = [persist.tile([128, DC], bf16, tag=f"wtv{k}", name=f"wtv{k}") for k in range(NK)]
            xkt = [persist.tile([128, S], bf16, tag=f"xkt{k}", name=f"xkt{k}") for k in range(NK)]

            # ---- helpers ---------------------------------------------------
            def dma_x_chunk(pool, xdram, tag, n):
                tiles = []
                for k in range(NK):
                    t_ = pool.tile([128, 512], bf16, tag=f"{tag}{k}", name=f"{tag}{k}")
                    nc.sync.dma_start(
                        t_[: ksz(k), :],
                        xdram[k * 128: k * 128 + ksz(k), n * 512:(n + 1) * 512],
                    )
                    tiles.append(t_)
                return tiles

            def chain_q(p, n, xt):
                ps = ppps.tile([128, 512], fp32, tag="pp")
                for k in range(NK):
                    nc.tensor.matmul(
                        ps[:], wtq[k][: ksz(k), p * 128:(p + 1) * 128],
                        xt[k][: ksz(k), :],
                        start=(k == 0), stop=(k == NK - 1),
                    )
                nc.vector.tensor_copy(qht[p][:, n * 512:(n + 1) * 512], ps[:])

            def chain_k(p, c):
                ps = ppps.tile([128, 512], fp32, tag="pp")
                for k in range(NK):
                    nc.tensor.matmul(
                        ps[:], wtk[k][: ksz(k), p * 128:(p + 1) * 128],
                        xkt[k][: ksz(k), c * 512:(c + 1) * 512],
                        start=(k == 0), stop=(k == NK - 1),
                    )
                nc.vector.tensor_copy(kht[p][:, c * 512:(c + 1) * 512], ps[:])

            def chain_v(t, xt, p0, p1):
                """V projection for t-block t, pairs [p0, p1) -> vh slots."""
                npair = p1 - p0
                col = t * 128 % 512
                ps = ppps.tile([128, 512], fp32, tag="pp")
                for k in range(NK):
                    nc.tensor.matmul(
                        ps[:, : npair * 128],
                        xt[k][: ksz(k), col: col + 128],
                        wtv[k][: ksz(k), p0 * 128: p1 * 128],
                        start=(k == 0), stop=(k == NK - 1),
                    )
                for p in range(p0, p1):
                    dst = vh[p][:, t * 2 * VW: t * 2 * VW + 2 * VW].rearrange(
                        "p (h c) -> p h c", c=VW
                    )[:, :, 0:DK]
                    src = ps[:, (p - p0) * 128: (p - p0 + 1) * 128].rearrange(
                        "p (h c) -> p h c", c=DK
                    )
                    nc.vector.tensor_copy(dst, src)

            c_osb = {}

            def chain_c(qt, nd):
                """Output projection for row block qt, 512-col half nd."""
                ps = ppps.tile([128, 512], fp32, tag="pp")
                for p in range(NP):
                    nc.tensor.matmul(
                        ps[:], ot[p][:, qt * 128:(qt + 1) * 128],
                        wo_sb[:, p * DOUT + nd * 512: p * DOUT + (nd + 1) * 512],
                        start=(p == 0), stop=(p == NP - 1),
                    )
                if nd == 0:
                    c_osb[qt] = cpool.tile([128, DOUT], fp32, tag="osb", name="osb")
                osb = c_osb[qt]
                nc.vector.tensor_copy(osb[:, nd * 512:(nd + 1) * 512], ps[:])
                if nd == ND - 1:
                    del c_osb[qt]
                    nc.sync.dma_start(outp[qt * 128:(qt + 1) * 128, :], osb[:])

            def normalize(p, n, acc):
                # bf16 evict frees the single PSUM accumulator fast (~0.7us)
                osb = opool.tile([128, 1024], bf16, tag="nosb")
                nc.vector.tensor_copy(osb[0:65, :], acc[0:65, :])
                den = bpool.tile([128, 1024], bf16, tag="den")
                nc.sync.dma_start(den[0:1, :], osb[64:65, :])
                denf = bpool.tile([128, 1024], fp32, tag="denf")
                nc.vector.tensor_copy(denf[0:1, :], den[0:1, :])
                rd = bpool.tile([128, 1024], fp32, tag="rd")
                nc.vector.reciprocal_approx_fast(rd[0:1, :], denf[0:1, :])
                bca = bpool.tile([64, 1024], fp32, tag="bca")
                nc.gpsimd.partition_broadcast(bca[:], rd[0:1, :], channels=64)
                qof = n * 512
                nc.vector.tensor_mul(
                    ot[p][0:64, qof: qof + 512], osb[0:64, 0:512], bca[:, 0:512]
                )
                tmpb = bpool.tile([64, 512], bf16, tag="tmpb")
                nc.vector.tensor_mul(tmpb[:], osb[0:64, 512:1024], bca[:, 512:1024])
                nc.sync.dma_start(ot[p][64:128, qof: qof + 512], tmpb[:])

            # ---- background queue (credit-paced PE work) -------------------
            bg = deque()
            bg_pair = [deque() for _ in range(NP)]
            credit = [0.0]

            def run_bg(rate, cap=4500.0):
                credit[0] = min(credit[0] + rate, cap)
                while bg and (credit[0] > 0 or bg[0][0] == 0):
                    cost, thunk = bg.popleft()
                    thunk()
                    credit[0] -= cost

            def refill(p):
                src = bg_pair[p]
                while src and len(bg) < 8:
                    bg.append(src.popleft())

            def drain_all():
                for p in range(NP):
                    while bg_pair[p]:
                        bg.append(bg_pair[p].popleft())
                while bg:
                    _, thunk = bg.popleft()
                    thunk()

            store = {}

            def mk_dma(pool, xdram, tag, n, key):
                def f():
                    store[key] = dma_x_chunk(pool, xdram, tag, n)
                return (0, f)

            def mk_q(p, n):
                return (NK * 512, lambda: chain_q(p, n, store[("q", p, n)]))

            def mk_k(p, c):
                return (NK * 512, lambda: chain_k(p, c, store[("k", p, c)]))

            def mk_v(t, key, p0, p1):
                return (NK * 128 * (p1 - p0), lambda: chain_v(t, store[key], p0, p1))

            # ---- prologue --------------------------------------------------
            for k in range(NK):
                nc.sync.dma_start(wtk[k][: ksz(k), :], wk[k * 128: k * 128 + ksz(k), :])
            for k in range(NK):
                nc.sync.dma_start(wtq[k][: ksz(k), :], wq[k * 128: k * 128 + ksz(k), :])
            # resident xk, chunk-column-major so K(0,c0) starts early
            for c in range(NQ):
                for k in range(NK):
                    nc.sync.dma_start(
                        xkt[k][: ksz(k), c * 512:(c + 1) * 512],
                        xk[k * 128: k * 128 + ksz(k), c * 512:(c + 1) * 512],
                    )
            store[("q", 0)] = dma_x_chunk(xqp, xq, "xq", 0)
            for k in range(NK):
                nc.sync.dma_start(wtv[k][: ksz(k), :], wv[k * 128: k * 128 + ksz(k), :])
            store[("va", 0)] = dma_x_chunk(xvp, xv, "xv", 0)
            nc.sync.dma_start(eye[:], eyec[:])
            for p in range(NP):
                nc.vector.memset(vh[p][:], 1.0)

            chain_k(0, 0)
            chain_k(0, 1)
            chain_q(0, 0, store[("q", 0)])
            chain_k(0, 2)
            chain_k(0, 3)
            for t in range(4):
                chain_v(t, store[("va", 0)], 0, 1)

            # ---- background schedules per pair -----------------------------
            def mk_q(p, key):
                return (NK * 512, lambda: chain_q(p, key[1], store[key]))

            def mk_k(p, c):
                return (NK * 512, lambda: chain_k(p, c))

            def mk_v(t, key, p0, p1):
                return (NK * 128 * (p1 - p0), lambda: chain_v(t, store[key], p0, p1))

            # pair 0: V(p0) t4..15 + Q(0,n1..3)/Q(1,*) interleaved by deadline,
            # then K(1), then V(pairs1-3) t0..7.
            b0 = bg_pair[0]
            b0.append(mk_dma(xvp, xv, "xv", 1, ("va", 1)))
            b0.append(mk_dma(xvp, xv, "xv", 2, ("va", 2)))
            for t in range(4, 8):
                b0.append(mk_v(t, ("va", 1), 0, 1))
            b0.append(mk_q(1, ("q", 0)))
            for t in range(8, 12):
                b0.append(mk_v(t, ("va", 2), 0, 1))
            b0.append(mk_dma(xvp, xv, "xv", 3, ("va", 3)))
            b0.append(mk_dma(xqp, xq, "xq", 1, ("q", 1)))
            b0.append(mk_v(12, ("va", 3), 0, 1))
            b0.append(mk_v(13, ("va", 3), 0, 1))
            b0.append(mk_q(0, ("q", 1)))
            b0.append(mk_v(14, ("va", 3), 0, 1))
            b0.append(mk_v(15, ("va", 3), 0, 1))
            b0.append(mk_q(1, ("q", 1)))
            b0.append(mk_dma(xqp, xq, "xq", 2, ("q", 2)))
            b0.append(mk_q(0, ("q", 2)))
            b0.append(mk_q(1, ("q", 2)))
            b0.append(mk_dma(xqp, xq, "xq", 3, ("q", 3)))
            b0.append(mk_q(0, ("q", 3)))
            b0.append(mk_q(1, ("q", 3)))
            for c in range(4):
                b0.append(mk_k(1, c))
            b0.append(mk_dma(xvp, xv, "xv", 0, ("vb", 0)))
            b0.append(mk_dma(xvp, xv, "xv", 1, ("vb", 1)))
            for t in range(0, 4):
                b0.append(mk_v(t, ("vb", 0), 1, 4))
            for t in range(4, 8):
                b0.append(mk_v(t, ("vb", 1), 1, 4))

            # pair 1: V(pairs1-3) t8..15 (own-pair deadlines), wo, K(2),
            # then xq pass-2 first half: Q(2..3, n0..1).
            b1 = bg_pair[1]
            b1.append(mk_dma(xvp, xv, "xv", 2, ("vb", 2)))
            b1.append(mk_dma(xvp, xv, "xv", 3, ("vb", 3)))
            for t in range(8, 12):
                b1.append(mk_v(t, ("vb", 2), 1, 4))
            for t in range(12, 16):
                b1.append(mk_v(t, ("vb", 3), 1, 4))

            def dma_wo():
                for p in range(NP):
                    nc.sync.dma_start(
                        wo_sb[:, p * DOUT:(p + 1) * DOUT], wo[p * 128:(p + 1) * 128, :]
                    )
            b1.append((0, dma_wo))
            for c in range(4):
                b1.append(mk_k(2, c))
            b1.append(mk_dma(xqp, xq, "xq", 0, ("q2", 0)))
            b1.append(mk_dma(xqp, xq, "xq", 1, ("q2", 1)))
            b1.append(mk_q(2, ("q2", 0)))
            b1.append(mk_q(3, ("q2", 0)))
            b1.append(mk_q(2, ("q2", 1)))
            b1.append(mk_q(3, ("q2", 1)))

            # pair 2: xq pass-2 second half + K(3)
            b2 = bg_pair[2]
            b2.append(mk_dma(xqp, xq, "xq", 2, ("q2", 2)))
            b2.append(mk_dma(xqp, xq, "xq", 3, ("q2", 3)))
            b2.append(mk_q(2, ("q2", 2)))
            b2.append(mk_q(3, ("q2", 2)))
            b2.append(mk_q(2, ("q2", 3)))
            b2.append(mk_q(3, ("q2", 3)))
            for c in range(4):
                b2.append(mk_k(3, c))

            # pair 3: output-projection chains are queued by normalize()

            # ---- main attention stream -------------------------------------
            iters = [(p, n, t) for p in range(NP) for n in range(NQ) for t in range(NT)]
            NIT = len(iters)
            e_buf = {}
            acc_buf = {}

            def emit_sc_exp(i):
                p, n, t = iters[i]
                sc = scps.tile([128, 1024], fp32, tag="sc")
                # row-tiled score matmuls: head A on PE tile (0,0), B on (64,0)
                nc.tensor.matmul(
                    sc[:, 0:512],
                    kht[p][0:64, t * 128:(t + 1) * 128],
                    qht[p][0:64, n * 512:(n + 1) * 512],
                    start=True, stop=True,
                )
                nc.tensor.matmul(
                    sc[:, 512:1024],
                    kht[p][64:128, t * 128:(t + 1) * 128],
                    qht[p][64:128, n * 512:(n + 1) * 512],
                    start=True, stop=True,
                )
                e = epool.tile([128, 1024], bf16, tag="e")
                nc.scalar.activation(
                    e[:], sc[:], mybir.ActivationFunctionType.Exp, scale=scale
                )
                off = t * 128 - n * 512
                if 0 <= off < 512:
                    nc.vector.tensor_mul(e[:, off: off + 128], e[:, off: off + 128], eye[:])
                    nc.vector.tensor_mul(
                        e[:, 512 + off: 512 + off + 128],
                        e[:, 512 + off: 512 + off + 128], eye[:],
                    )
                e_buf[i] = e

            def emit_pv(j):
                p, n, t = iters[j]
                if t == 0:
                    acc_buf[(p, n)] = acps.tile([128, 1024], fp32, tag="acc", name="acc")
                acc = acc_buf[(p, n)]
                e = e_buf.pop(j)
                vbase = t * 2 * VW
                nc.tensor.matmul(
                    acc[0:65, 0:512],
                    vh[p][:, vbase: vbase + 65],
                    e[:, 0:512],
                    start=(t == 0), stop=(t == NT - 1),
                )
                nc.tensor.matmul(
                    acc[0:65, 512:1024],
                    vh[p][:, vbase + VW: vbase + VW + 65],
                    e[:, 512:1024],
                    start=(t == 0), stop=(t == NT - 1),
                )
                if t == NT - 1:
                    normalize(p, n, acc_buf.pop((p, n)))
                    if p == NP - 1:
                        for qt in range(n * 4, n * 4 + 4):
                            for nd in range(ND):
                                bg_pair[3].append(
                                    (NP * 512, (lambda qt_=qt, nd_=nd: chain_c(qt_, nd_)))
                                )

            # 2-iteration batches: [sc(i), sc(i+1)] then [pv(i-4), pv(i-3)]
            # so the PE switches tiling mode once per iteration on average.
            for i in range(0, NIT, 2):
                p, n, t = iters[i]
                emit_sc_exp(i)
                emit_sc_exp(i + 1)
                if i >= LAG + 1:
                    emit_pv(i - LAG - 1)
                    emit_pv(i - LAG)
                refill(p)
                run_bg(2 * (1400.0 if p <= 1 else 1150.0), cap=7000.0)
            for j in range(NIT - LAG - 1, NIT):
                emit_pv(j)
                run_bg(1400.0, cap=7000.0)
            drain_all()

    nc.compile()
    return nc


def _bf16(a):
    return np.ascontiguousarray(a).astype(ml_dtypes.bfloat16)


def _prep_core_inputs(q, k, v, Wq, bq, Wk, bk, Wv, bv, Wo, aug_bias):
    """Per-core host-side slicing/transposition. Returns list of 8 dicts."""
    eyec = _bf16(1.0 - np.eye(128, dtype=np.float32))
    maps = []
    for c in range(N_CORES):
        b = c // 2
        h0 = (c % 2) * HEADS_PER_CORE
        r0, r1 = h0 * DK, (h0 + HEADS_PER_CORE) * DK
        m = {}
        for name, x in (("xq", q[b]), ("xk", k[b]), ("xv", v[b])):
            xt = x.T  # [D, S]
            if aug_bias:
                xt = np.concatenate([xt, np.ones((1, S), np.float32)], axis=0)
            m[name] = _bf16(xt)
        for name, W, bias in (("wq", Wq, bq), ("wk", Wk, bk), ("wv", Wv, bv)):
            wt = W[r0:r1, :].T  # [D, DC]
            if aug_bias:
                wt = np.concatenate([wt, bias[None, r0:r1]], axis=0)
            m[name] = _bf16(wt)
        m["wo"] = _bf16(Wo[:, r0:r1].T)  # [DC, D]
        m["eyec"] = eyec
        maps.append(m)
    return maps


_PROGRAM_CACHE = {}


def _get_program(aug_bias):
    if aug_bias not in _PROGRAM_CACHE:
        _PROGRAM_CACHE[aug_bias] = build_attention_core(
            S=S, DIN=D, NH=HEADS_PER_CORE, DOUT=D, aug_bias=aug_bias
        )
    return _PROGRAM_CACHE[aug_bias]


def _reference_fallback(q, k, v, Wq, bq, Wk, bk, Wv, bv, Wo, bo, mask):
    """Pure-numpy fallback for unexpected mask patterns."""
    out = np.empty((B, S, D), np.float32)
    msk = np.broadcast_to(mask.reshape(mask.shape[-2], mask.shape[-1]), (S, S))
    for b in range(B):
        qh = (q[b] @ Wq.T + bq).reshape(S, H, DK).transpose(1, 0, 2)
        kh = (k[b] @ Wk.T + bk).reshape(S, H, DK).transpose(1, 0, 2)
        vh = (v[b] @ Wv.T + bv).reshape(S, H, DK).transpose(1, 0, 2)
        acc = np.empty((H, S, DK), np.float32)
        for h in range(H):
            s = (qh[h] @ kh[h].T) / np.float32(np.sqrt(DK))
            s = np.where(msk == 0, np.finfo(np.float32).min, s)
            s = s - s.max(axis=-1, keepdims=True)
            e = np.exp(s)
            p = e / e.sum(axis=-1, keepdims=True)
            acc[h] = p @ vh[h]
        o = acc.transpose(1, 0, 2).reshape(S, D)
        out[b] = o @ Wo.T + bo
    return out


def kernel(q, k, v, Wq, bq, Wk, bk, Wv, bv, Wo, bo, mask, _trace=False):
    from concourse.bass_utils import run_bass_kernel_spmd

    q = np.asarray(q, np.float32)
    k = np.asarray(k, np.float32)
    v = np.asarray(v, np.float32)
    Wq, bq = np.asarray(Wq, np.float32), np.asarray(bq, np.float32)
    Wk, bk = np.asarray(Wk, np.float32), np.asarray(bk, np.float32)
    Wv, bv = np.asarray(Wv, np.float32), np.asarray(bv, np.float32)
    Wo, bo = np.asarray(Wo, np.float32), np.asarray(bo, np.float32)
    mask = np.asarray(mask)

    expected_mask = 1 - np.eye(S, dtype=np.int32)
    if not np.array_equal(mask.reshape(-1, S, S)[0].astype(np.int32), expected_mask):
        return _reference_fallback(q, k, v, Wq, bq, Wk, bk, Wv, bv, Wo, bo, mask)

    aug_bias = bool(np.any(bq) or np.any(bk) or np.any(bv))
    nc = _get_program(aug_bias)
    in_maps = _prep_core_inputs(q, k, v, Wq, bq, Wk, bk, Wv, bv, Wo, aug_bias)
    res = run_bass_kernel_spmd(
        nc, in_maps, core_ids=list(range(N_CORES)), trace=_trace
    )
    out = np.empty((B, S, D), np.float32)
    for b in range(B):
        out[b] = res.results[2 * b]["outp"] + res.results[2 * b + 1]["outp"] + bo
    if _trace:
        kernel.last_results = res
    return out


# revision 12
# speedup vs baseline: 1.1680x; 1.0507x over previous
"""Diagonal-masked multi-head self-attention on 8 TRN2 NeuronCores.

Sharding: core c handles batch b = c // 2 and heads h0 = (c % 2) * 8 .. +8
(data parallel on B=4, tensor parallel over the 16 heads).  Each core
computes a partial output [S, D]; the host sums the two half-head partials
per batch and adds the output bias.

Per-core design (v2, software-pipelined):
  The softmax exp on the Scalar engine (256 tiles x [128,1024] @ ~1.1us) is
  the hard floor (~290us), so the whole kernel is paced by the exp stream:

  - Attention runs as one flat stream of 256 iterations (pair-major, then
    q-chunk, then t-block).  Each iteration emits the two score matmuls
    (row-tiled: the K=64 head halves run concurrently on PE array tiles
    (0,0)/(64,0)), the exp, the (rare) diagonal-mask multiply, and the PV
    matmuls of the iteration LAG=3 behind (so exp latency never stalls the
    PE).
  - All projection work (Q/K per pair, V, and the output projection) is
    chopped into ~8-matmul chains and drip-fed into the same instruction
    stream as credit-paced "background" PE work, so phases fully overlap:
    the PE projects pair p+1 while the Scalar engine exps pair p.
  - x inputs are streamed from DRAM in per-chunk tile sets (double
    buffered, DMA emitted one chain ahead); V is projected in two column
    passes (pair 0 at N=128 so attention starts early, pairs 1-3 at N=384
    in background).
  - PSUM (8 banks): scores 2x[128,1024], accumulator 1x[128,1024]
    (evicted fp32 to SBUF right after each 16-t accumulation), projection
    chains 2x[128,512].
  - Scalar does ONLY exp.  All PSUM evictions are Vector copies.  The
    softmax denominator falls out of a ones column in the V weights
    (row 64 of the accumulator); reciprocal+broadcast+multiply run on
    Vector/GpSimd off the critical path.
"""

import numpy as np
import ml_dtypes

B, S, D, H = 4, 2048, 1024, 16
DK = D // H
N_CORES = 8
HEADS_PER_CORE = H // 2


def build_attention_core(S=2048, DIN=1024, NH=8, DOUT=1024, aug_bias=False):
    from collections import deque

    import concourse.bacc as bacc
    import concourse.mybir as mybir
    import concourse.tile as tile

    fp32 = mybir.dt.float32
    bf16 = mybir.dt.bfloat16

    NP = NH // 2              # head pairs per core (4)
    DC = NH * DK              # concat head dim on this core (512)
    NT = S // 128             # t tiles (16)
    NQ = S // 512             # q chunks (4)
    KA = DIN + 1 if aug_bias else DIN
    NK = (KA + 127) // 128    # contraction tiles for projections
    VW = 66                   # per-head V slot: V(64) + ones(1) + pad(1)
    LAG = 3                   # pv trails sc/exp by LAG iterations
    ND = DOUT // 512

    nc = bacc.Bacc(None, target_bir_lowering=False, debug=False)

    xq = nc.dram_tensor("xq", [KA, S], bf16, kind="ExternalInput")
    xk = nc.dram_tensor("xk", [KA, S], bf16, kind="ExternalInput")
    xv = nc.dram_tensor("xv", [KA, S], bf16, kind="ExternalInput")
    wq = nc.dram_tensor("wq", [KA, DC], bf16, kind="ExternalInput")
    wk = nc.dram_tensor("wk", [KA, DC], bf16, kind="ExternalInput")
    wv = nc.dram_tensor("wv", [KA, DC], bf16, kind="ExternalInput")
    wo = nc.dram_tensor("wo", [DC, DOUT], bf16, kind="ExternalInput")
    eyec = nc.dram_tensor("eyec", [128, 128], bf16, kind="ExternalInput")
    outp = nc.dram_tensor("outp", [S, DOUT], fp32, kind="ExternalOutput")

    def ksz(k):
        return min(128, KA - k * 128)

    scale = float(1.0 / np.sqrt(DK))

    with tile.TileContext(nc) as tc:
        with (
            tc.tile_pool(name="persist", bufs=1) as persist,
            tc.tile_pool(name="xqp", bufs=2) as xqp,
            tc.tile_pool(name="xvp", bufs=2) as xvp,
            tc.tile_pool(name="epool", bufs=LAG + 4) as epool,
            tc.tile_pool(name="opool", bufs=2) as opool,
            tc.tile_pool(name="bpool", bufs=1) as bpool,
            tc.tile_pool(name="cpool", bufs=2) as cpool,
            tc.tile_pool(name="scps", bufs=2, space="PSUM") as scps,
            tc.tile_pool(name="acps", bufs=1, space="PSUM") as acps,
            tc.tile_pool(name="ppps", bufs=2, space="PSUM") as ppps,
        ):
            # ---- persistent SBUF ------------------------------------------
            qht = [persist.tile([128, S], bf16, tag=f"qht{p}", name=f"qht{p}") for p in range(NP)]
            kht = [persist.tile([128, S], bf16, tag=f"kht{p}", name=f"kht{p}") for p in range(NP)]
            vh = [persist.tile([128, NT * 2 * VW], bf16, tag=f"vh{p}", name=f"vh{p}") for p in range(NP)]
            ot = [persist.tile([128, S], bf16, tag=f"ot{p}", name=f"ot{p}") for p in range(NP)]
            eye = persist.tile([128, 128], bf16, tag="eye")
            wo_sb = persist.tile([128, NP * DOUT], bf16, tag="wo")
            NKF = KA // 128           # full 128-row contraction tiles
            wtq_all = persist.tile([128, NK * DC], bf16, tag="wtq", name="wtq_all")
            wtk_all = persist.tile([128, NK * DC], bf16, tag="wtk", name="wtk_all")
            wtv_all = persist.tile([128, NK * DC], bf16, tag="wtv", name="wtv_all")
            wtq = [wtq_all[:, k * DC:(k + 1) * DC] for k in range(NK)]
            wtk = [wtk_all[:, k * DC:(k + 1) * DC] for k in range(NK)]
            wtv = [wtv_all[:, k * DC:(k + 1) * DC] for k in range(NK)]
            xkt = [persist.tile([128, S], bf16, tag=f"xkt{k}", name=f"xkt{k}") for k in range(NK)]

            def dma_w_all(dst_all, wdram):
                # one DMA for the full [KA(full part), DC] weight block
                nc.sync.dma_start(
                    dst_all[:, 0: NKF * DC].rearrange("p (k c) -> p k c", c=DC),
                    wdram[0: NKF * 128, :].rearrange("(k p) c -> p k c", p=128),
                )
                if KA > NKF * 128:  # aug bias row
                    nc.sync.dma_start(
                        dst_all[0:1, NKF * DC:(NKF + 1) * DC], wdram[NKF * 128:, :]
                    )

            # ---- helpers ---------------------------------------------------
            def dma_x_chunk(pool, xdram, tag, n):
                big = pool.tile([128, NK * 512], bf16, tag=tag, name=tag)
                for k0 in range(0, NKF, 2):
                    nc.sync.dma_start(
                        big[:, k0 * 512:(k0 + 2) * 512].rearrange(
                            "p (k c) -> p k c", c=512
                        ),
                        xdram[k0 * 128:(k0 + 2) * 128, n * 512:(n + 1) * 512]
                        .rearrange("(k p) c -> p k c", p=128),
                    )
                if KA > NKF * 128:
                    nc.sync.dma_start(
                        big[0:1, NKF * 512:(NKF + 1) * 512],
                        xdram[NKF * 128:, n * 512:(n + 1) * 512],
                    )
                return [big[:, k * 512:(k + 1) * 512] for k in range(NK)]

            def chain_q(p, n, xt):
                ps = ppps.tile([128, 512], fp32, tag="pp")
                for k in range(NK):
                    nc.tensor.matmul(
                        ps[:], wtq[k][: ksz(k), p * 128:(p + 1) * 128],
                        xt[k][: ksz(k), :],
                        start=(k == 0), stop=(k == NK - 1),
                    )
                nc.vector.tensor_copy(qht[p][:, n * 512:(n + 1) * 512], ps[:])

            def chain_k(p, c):
                ps = ppps.tile([128, 512], fp32, tag="pp")
                for k in range(NK):
                    nc.tensor.matmul(
                        ps[:], wtk[k][: ksz(k), p * 128:(p + 1) * 128],
                        xkt[k][: ksz(k), c * 512:(c + 1) * 512],
                        start=(k == 0), stop=(k == NK - 1),
                    )
                nc.vector.tensor_copy(kht[p][:, c * 512:(c + 1) * 512], ps[:])

            def chain_v(t, xt, p0, p1):
                """V projection for t-block t, pairs [p0, p1) -> vh slots."""
                npair = p1 - p0
                col = t * 128 % 512
                ps = ppps.tile([128, 512], fp32, tag="pp")
                for k in range(NK):
                    nc.tensor.matmul(
                        ps[:, : npair * 128],
                        xt[k][: ksz(k), col: col + 128],
                        wtv[k][: ksz(k), p0 * 128: p1 * 128],
                        start=(k == 0), stop=(k == NK - 1),
                    )
                for p in range(p0, p1):
                    dst = vh[p][:, t * 2 * VW: t * 2 * VW + 2 * VW].rearrange(
                        "p (h c) -> p h c", c=VW
                    )[:, :, 0:DK]
                    src = ps[:, (p - p0) * 128: (p - p0 + 1) * 128].rearrange(
                        "p (h c) -> p h c", c=DK
                    )
                    nc.vector.tensor_copy(dst, src)

            c_osb = {}

            def chain_c(qt, nd):
                """Output projection for row block qt, 512-col half nd."""
                ps = ppps.tile([128, 512], fp32, tag="pp")
                for p in range(NP):
                    nc.tensor.matmul(
                        ps[:], ot[p][:, qt * 128:(qt + 1) * 128],
                        wo_sb[:, p * DOUT + nd * 512: p * DOUT + (nd + 1) * 512],
                        start=(p == 0), stop=(p == NP - 1),
                    )
                if nd == 0:
                    c_osb[qt] = cpool.tile([128, DOUT], fp32, tag="osb", name="osb")
                osb = c_osb[qt]
                nc.vector.tensor_copy(osb[:, nd * 512:(nd + 1) * 512], ps[:])
                if nd == ND - 1:
                    del c_osb[qt]
                    nc.sync.dma_start(outp[qt * 128:(qt + 1) * 128, :], osb[:])

            def normalize(p, n, acc):
                # bf16 numerator evict + fp32 denominator DMA free the single
                # PSUM accumulator in ~1.3us
                osb = opool.tile([128, 1024], bf16, tag="nosb")
                nc.vector.tensor_copy(osb[0:65, :], acc[0:65, :])
                den = bpool.tile([128, 1024], bf16, tag="den")
                nc.sync.dma_start(den[0:1, :], osb[64:65, :])
                denf = bpool.tile([128, 1024], fp32, tag="denf")
                nc.vector.tensor_copy(denf[0:1, :], den[0:1, :])
                rd = bpool.tile([128, 1024], fp32, tag="rd")
                nc.vector.reciprocal_approx_fast(rd[0:1, :], denf[0:1, :])
                bca = bpool.tile([64, 1024], fp32, tag="bca")
                nc.gpsimd.partition_broadcast(bca[:], rd[0:1, :], channels=64)
                qof = n * 512
                nc.vector.tensor_mul(
                    ot[p][0:64, qof: qof + 512], osb[0:64, 0:512], bca[:, 0:512]
                )
                tmpb = bpool.tile([64, 512], bf16, tag="tmpb")
                nc.vector.tensor_mul(tmpb[:], osb[0:64, 512:1024], bca[:, 512:1024])
                nc.sync.dma_start(ot[p][64:128, qof: qof + 512], tmpb[:])

            # ---- background queue (credit-paced PE work) -------------------
            bg = deque()
            bg_pair = [deque() for _ in range(NP)]
            credit = [0.0]

            def run_bg(rate, cap=4500.0):
                credit[0] = min(credit[0] + rate, cap)
                while bg and (credit[0] > 0 or bg[0][0] == 0):
                    cost, thunk = bg.popleft()
                    thunk()
                    credit[0] -= cost

            def refill(p):
                src = bg_pair[p]
                while src and len(bg) < 8:
                    bg.append(src.popleft())

            def drain_all():
                for p in range(NP):
                    while bg_pair[p]:
                        bg.append(bg_pair[p].popleft())
                while bg:
                    _, thunk = bg.popleft()
                    thunk()

            store = {}

            def mk_dma(pool, xdram, tag, n, key):
                def f():
                    store[key] = dma_x_chunk(pool, xdram, tag, n)
                return (0, f)

            def mk_q(p, n):
                return (NK * 512, lambda: chain_q(p, n, store[("q", p, n)]))

            def mk_k(p, c):
                return (NK * 512, lambda: chain_k(p, c, store[("k", p, c)]))

            def mk_v(t, key, p0, p1):
                return (NK * 128 * (p1 - p0), lambda: chain_v(t, store[key], p0, p1))

            # ---- prologue --------------------------------------------------
            def dma_xk_half(h):
                for k in range(NK):
                    nc.sync.dma_start(
                        xkt[k][: ksz(k), h * 1024:(h + 1) * 1024],
                        xk[k * 128: k * 128 + ksz(k), h * 1024:(h + 1) * 1024],
                    )
            dma_w_all(wtk_all, wk)
            dma_xk_half(0)
            store[("q", 0)] = dma_x_chunk(xqp, xq, "xq", 0)
            dma_w_all(wtq_all, wq)
            dma_xk_half(1)
            dma_w_all(wtv_all, wv)
            store[("va", 0)] = dma_x_chunk(xvp, xv, "xv", 0)
            nc.sync.dma_start(eye[:], eyec[:])
            for p in range(NP):
                nc.vector.memset(vh[p][:], 1.0)

            chain_k(0, 0)
            chain_k(0, 1)
            chain_q(0, 0, store[("q", 0)])
            chain_k(0, 2)
            chain_k(0, 3)
            for t in range(4):
                chain_v(t, store[("va", 0)], 0, 1)

            # ---- background schedules per pair -----------------------------
            def mk_q(p, key):
                return (NK * 512, lambda: chain_q(p, key[1], store[key]))

            def mk_k(p, c):
                return (NK * 512, lambda: chain_k(p, c))

            def mk_v(t, key, p0, p1):
                return (NK * 128 * (p1 - p0), lambda: chain_v(t, store[key], p0, p1))

            # pair 0: V(p0) t4..15 + Q(0,n1..3)/Q(1,*) interleaved by deadline,
            # then K(1), then V(pairs1-3) t0..7.
            b0 = bg_pair[0]
            b0.append(mk_dma(xvp, xv, "xv", 1, ("va", 1)))
            b0.append(mk_dma(xvp, xv, "xv", 2, ("va", 2)))
            for t in range(4, 8):
                b0.append(mk_v(t, ("va", 1), 0, 1))
            b0.append(mk_q(1, ("q", 0)))
            for t in range(8, 12):
                b0.append(mk_v(t, ("va", 2), 0, 1))
            b0.append(mk_dma(xvp, xv, "xv", 3, ("va", 3)))
            b0.append(mk_dma(xqp, xq, "xq", 1, ("q", 1)))
            b0.append(mk_v(12, ("va", 3), 0, 1))
            b0.append(mk_v(13, ("va", 3), 0, 1))
            b0.append(mk_q(0, ("q", 1)))
            b0.append(mk_v(14, ("va", 3), 0, 1))
            b0.append(mk_v(15, ("va", 3), 0, 1))
            b0.append(mk_q(1, ("q", 1)))
            b0.append(mk_dma(xqp, xq, "xq", 2, ("q", 2)))
            b0.append(mk_q(0, ("q", 2)))
            b0.append(mk_q(1, ("q", 2)))
            b0.append(mk_dma(xqp, xq, "xq", 3, ("q", 3)))
            b0.append(mk_q(0, ("q", 3)))
            b0.append(mk_q(1, ("q", 3)))
            for c in range(4):
                b0.append(mk_k(1, c))
            b0.append(mk_dma(xvp, xv, "xv", 0, ("vb", 0)))
            b0.append(mk_dma(xvp, xv, "xv", 1, ("vb", 1)))
            for t in range(0, 4):
                b0.append(mk_v(t, ("vb", 0), 1, 4))
            for t in range(4, 8):
                b0.append(mk_v(t, ("vb", 1), 1, 4))

            # pair 1: V(pairs1-3) t8..15 (own-pair deadlines), wo, K(2),
            # then xq pass-2 first half: Q(2..3, n0..1).
            b1 = bg_pair[1]
            b1.append(mk_dma(xvp, xv, "xv", 2, ("vb", 2)))
            b1.append(mk_dma(xvp, xv, "xv", 3, ("vb", 3)))
            for t in range(8, 12):
                b1.append(mk_v(t, ("vb", 2), 1, 4))
            for t in range(12, 16):
                b1.append(mk_v(t, ("vb", 3), 1, 4))

            def dma_wo():
                nc.sync.dma_start(
                    wo_sb[:].rearrange("p (g c) -> p g c", c=DOUT),
                    wo[:].rearrange("(g p) c -> p g c", p=128),
                )
            b1.append((0, dma_wo))
            for c in range(4):
                b1.append(mk_k(2, c))
            b1.append(mk_dma(xqp, xq, "xq", 0, ("q2", 0)))
            b1.append(mk_dma(xqp, xq, "xq", 1, ("q2", 1)))
            b1.append(mk_q(2, ("q2", 0)))
            b1.append(mk_q(3, ("q2", 0)))
            b1.append(mk_q(2, ("q2", 1)))
            b1.append(mk_q(3, ("q2", 1)))

            # pair 2: xq pass-2 second half + K(3)
            b2 = bg_pair[2]
            b2.append(mk_dma(xqp, xq, "xq", 2, ("q2", 2)))
            b2.append(mk_dma(xqp, xq, "xq", 3, ("q2", 3)))
            b2.append(mk_q(2, ("q2", 2)))
            b2.append(mk_q(3, ("q2", 2)))
            b2.append(mk_q(2, ("q2", 3)))
            b2.append(mk_q(3, ("q2", 3)))
            for c in range(4):
                b2.append(mk_k(3, c))

            # pair 3: output-projection chains are queued by normalize()

            # ---- main attention stream -------------------------------------
            iters = [(p, n, t) for p in range(NP) for n in range(NQ) for t in range(NT)]
            NIT = len(iters)
            e_buf = {}
            acc_buf = {}

            def emit_sc_exp(i):
                p, n, t = iters[i]
                sc = scps.tile([128, 1024], fp32, tag="sc")
                # row-tiled score matmuls: head A on PE tile (0,0), B on (64,0)
                nc.tensor.matmul(
                    sc[:, 0:512],
                    kht[p][0:64, t * 128:(t + 1) * 128],
                    qht[p][0:64, n * 512:(n + 1) * 512],
                    start=True, stop=True,
                )
                nc.tensor.matmul(
                    sc[:, 512:1024],
                    kht[p][64:128, t * 128:(t + 1) * 128],
                    qht[p][64:128, n * 512:(n + 1) * 512],
                    start=True, stop=True,
                )
                e = epool.tile([128, 1024], bf16, tag="e")
                nc.scalar.activation(
                    e[:], sc[:], mybir.ActivationFunctionType.Exp, scale=scale
                )
                off = t * 128 - n * 512
                if 0 <= off < 512:
                    nc.vector.tensor_mul(e[:, off: off + 128], e[:, off: off + 128], eye[:])
                    nc.vector.tensor_mul(
                        e[:, 512 + off: 512 + off + 128],
                        e[:, 512 + off: 512 + off + 128], eye[:],
                    )
                e_buf[i] = e

            def emit_pv(j):
                p, n, t = iters[j]
                if t == 0:
                    acc_buf[(p, n)] = acps.tile([128, 1024], fp32, tag="acc", name="acc")
                acc = acc_buf[(p, n)]
                e = e_buf.pop(j)
                vbase = t * 2 * VW
                nc.tensor.matmul(
                    acc[0:65, 0:512],
                    vh[p][:, vbase: vbase + 65],
                    e[:, 0:512],
                    start=(t == 0), stop=(t == NT - 1),
                )
                nc.tensor.matmul(
                    acc[0:65, 512:1024],
                    vh[p][:, vbase + VW: vbase + VW + 65],
                    e[:, 512:1024],
                    start=(t == 0), stop=(t == NT - 1),
                )
                if t == NT - 1:
                    normalize(p, n, acc_buf.pop((p, n)))
                    if p == NP - 1:
                        for qt in range(n * 4, n * 4 + 4):
                            for nd in range(ND):
                                bg_pair[3].append(
                                    (NP * 512, (lambda qt_=qt, nd_=nd: chain_c(qt_, nd_)))
                                )

            # 2-iteration batches: [sc(i), sc(i+1)] then [pv(i-4), pv(i-3)]
            # so the PE switches tiling mode once per iteration on average.
            for i in range(0, NIT, 2):
                p, n, t = iters[i]
                emit_sc_exp(i)
                emit_sc_exp(i + 1)
                if i >= LAG + 1:
                    emit_pv(i - LAG - 1)
                    emit_pv(i - LAG)
                refill(p)
                run_bg(2 * (1400.0 if p <= 1 else (1150.0 if p == 2 else 1050.0)), cap=7000.0)
            for j in range(NIT - LAG - 1, NIT):
                emit_pv(j)
                run_bg(1400.0, cap=7000.0)
            drain_all()

    nc.compile()
    return nc


def _bf16(a):
    return np.ascontiguousarray(a).astype(ml_dtypes.bfloat16)


def _prep_core_inputs(q, k, v, Wq, bq, Wk, bk, Wv, bv, Wo, aug_bias):
    """Per-core host-side slicing/transposition. Returns list of 8 dicts."""
    eyec = _bf16(1.0 - np.eye(128, dtype=np.float32))
    maps = []
    for c in range(N_CORES):
        b = c // 2
        h0 = (c % 2) * HEADS_PER_CORE
        r0, r1 = h0 * DK, (h0 + HEADS_PER_CORE) * DK
        m = {}
        for name, x in (("xq", q[b]), ("xk", k[b]), ("xv", v[b])):
            xt = x.T  # [D, S]
            if aug_bias:
                xt = np.concatenate([xt, np.ones((1, S), np.float32)], axis=0)
            m[name] = _bf16(xt)
        for name, W, bias in (("wq", Wq, bq), ("wk", Wk, bk), ("wv", Wv, bv)):
            wt = W[r0:r1, :].T  # [D, DC]
            if aug_bias:
                wt = np.concatenate([wt, bias[None, r0:r1]], axis=0)
            m[name] = _bf16(wt)
        m["wo"] = _bf16(Wo[:, r0:r1].T)  # [DC, D]
        m["eyec"] = eyec
        maps.append(m)
    return maps


_PROGRAM_CACHE = {}


def _get_program(aug_bias):
    if aug_bias not in _PROGRAM_CACHE:
        _PROGRAM_CACHE[aug_bias] = build_attention_core(
            S=S, DIN=D, NH=HEADS_PER_CORE, DOUT=D, aug_bias=aug_bias
        )
    return _PROGRAM_CACHE[aug_bias]


def _reference_fallback(q, k, v, Wq, bq, Wk, bk, Wv, bv, Wo, bo, mask):
    """Pure-numpy fallback for unexpected mask patterns."""
    out = np.empty((B, S, D), np.float32)
    msk = np.broadcast_to(mask.reshape(mask.shape[-2], mask.shape[-1]), (S, S))
    for b in range(B):
        qh = (q[b] @ Wq.T + bq).reshape(S, H, DK).transpose(1, 0, 2)
        kh = (k[b] @ Wk.T + bk).reshape(S, H, DK).transpose(1, 0, 2)
        vh = (v[b] @ Wv.T + bv).reshape(S, H, DK).transpose(1, 0, 2)
        acc = np.empty((H, S, DK), np.float32)
        for h in range(H):
            s = (qh[h] @ kh[h].T) / np.float32(np.sqrt(DK))
            s = np.where(msk == 0, np.finfo(np.float32).min, s)
            s = s - s.max(axis=-1, keepdims=True)
            e = np.exp(s)
            p = e / e.sum(axis=-1, keepdims=True)
            acc[h] = p @ vh[h]
        o = acc.transpose(1, 0, 2).reshape(S, D)
        out[b] = o @ Wo.T + bo
    return out


def kernel(q, k, v, Wq, bq, Wk, bk, Wv, bv, Wo, bo, mask, _trace=False):
    from concourse.bass_utils import run_bass_kernel_spmd

    q = np.asarray(q, np.float32)
    k = np.asarray(k, np.float32)
    v = np.asarray(v, np.float32)
    Wq, bq = np.asarray(Wq, np.float32), np.asarray(bq, np.float32)
    Wk, bk = np.asarray(Wk, np.float32), np.asarray(bk, np.float32)
    Wv, bv = np.asarray(Wv, np.float32), np.asarray(bv, np.float32)
    Wo, bo = np.asarray(Wo, np.float32), np.asarray(bo, np.float32)
    mask = np.asarray(mask)

    expected_mask = 1 - np.eye(S, dtype=np.int32)
    if not np.array_equal(mask.reshape(-1, S, S)[0].astype(np.int32), expected_mask):
        return _reference_fallback(q, k, v, Wq, bq, Wk, bk, Wv, bv, Wo, bo, mask)

    aug_bias = bool(np.any(bq) or np.any(bk) or np.any(bv))
    nc = _get_program(aug_bias)
    in_maps = _prep_core_inputs(q, k, v, Wq, bq, Wk, bk, Wv, bv, Wo, aug_bias)
    res = run_bass_kernel_spmd(
        nc, in_maps, core_ids=list(range(N_CORES)), trace=_trace
    )
    out = np.empty((B, S, D), np.float32)
    for b in range(B):
        out[b] = res.results[2 * b]["outp"] + res.results[2 * b + 1]["outp"] + bo
    if _trace:
        kernel.last_results = res
    return out


# revision 13
# speedup vs baseline: 1.1921x; 1.0206x over previous
"""Diagonal-masked multi-head self-attention on 8 TRN2 NeuronCores.

Sharding: core c handles batch b = c // 2 and heads h0 = (c % 2) * 8 .. +8
(data parallel on B=4, tensor parallel over the 16 heads).  Each core
computes a partial output [S, D]; the host sums the two half-head partials
per batch and adds the output bias.

Per-core design (v2, software-pipelined):
  The softmax exp on the Scalar engine (256 tiles x [128,1024] @ ~1.1us) is
  the hard floor (~290us), so the whole kernel is paced by the exp stream:

  - Attention runs as one flat stream of 256 iterations (pair-major, then
    q-chunk, then t-block).  Each iteration emits the two score matmuls
    (row-tiled: the K=64 head halves run concurrently on PE array tiles
    (0,0)/(64,0)), the exp, the (rare) diagonal-mask multiply, and the PV
    matmuls of the iteration LAG=3 behind (so exp latency never stalls the
    PE).
  - All projection work (Q/K per pair, V, and the output projection) is
    chopped into ~8-matmul chains and drip-fed into the same instruction
    stream as credit-paced "background" PE work, so phases fully overlap:
    the PE projects pair p+1 while the Scalar engine exps pair p.
  - x inputs are streamed from DRAM in per-chunk tile sets (double
    buffered, DMA emitted one chain ahead); V is projected in two column
    passes (pair 0 at N=128 so attention starts early, pairs 1-3 at N=384
    in background).
  - PSUM (8 banks): scores 2x[128,1024], accumulator 1x[128,1024]
    (evicted fp32 to SBUF right after each 16-t accumulation), projection
    chains 2x[128,512].
  - Scalar does ONLY exp.  All PSUM evictions are Vector copies.  The
    softmax denominator falls out of a ones column in the V weights
    (row 64 of the accumulator); reciprocal+broadcast+multiply run on
    Vector/GpSimd off the critical path.
"""

import numpy as np
import ml_dtypes

B, S, D, H = 4, 2048, 1024, 16
DK = D // H
N_CORES = 8
HEADS_PER_CORE = H // 2


def build_attention_core(S=2048, DIN=1024, NH=8, DOUT=1024, aug_bias=False):
    from collections import deque

    import concourse.bacc as bacc
    import concourse.mybir as mybir
    import concourse.tile as tile

    fp32 = mybir.dt.float32
    bf16 = mybir.dt.bfloat16

    NP = NH // 2              # head pairs per core (4)
    DC = NH * DK              # concat head dim on this core (512)
    NT = S // 128             # t tiles (16)
    NQ = S // 512             # q chunks (4)
    KA = DIN + 1 if aug_bias else DIN
    NK = (KA + 127) // 128    # contraction tiles for projections
    VW = 66                   # per-head V slot: V(64) + ones(1) + pad(1)
    LAG = 3                   # pv trails sc/exp by LAG iterations
    ND = DOUT // 512

    nc = bacc.Bacc(None, target_bir_lowering=False, debug=False)

    xq = nc.dram_tensor("xq", [KA, S], bf16, kind="ExternalInput")
    xk = nc.dram_tensor("xk", [KA, S], bf16, kind="ExternalInput")
    xv = nc.dram_tensor("xv", [KA, S], bf16, kind="ExternalInput")
    wq = nc.dram_tensor("wq", [KA, DC], bf16, kind="ExternalInput")
    wk = nc.dram_tensor("wk", [KA, DC], bf16, kind="ExternalInput")
    wv = nc.dram_tensor("wv", [KA, DC], bf16, kind="ExternalInput")
    wo = nc.dram_tensor("wo", [DC, DOUT], bf16, kind="ExternalInput")
    eyec = nc.dram_tensor("eyec", [128, 128], bf16, kind="ExternalInput")
    outp = nc.dram_tensor("outp", [S, DOUT], fp32, kind="ExternalOutput")

    def ksz(k):
        return min(128, KA - k * 128)

    scale = float(1.0 / np.sqrt(DK))

    with tile.TileContext(nc) as tc:
        with (
            tc.tile_pool(name="persist", bufs=1) as persist,
            tc.tile_pool(name="xqp", bufs=2) as xqp,
            tc.tile_pool(name="xvp", bufs=2) as xvp,
            tc.tile_pool(name="epool", bufs=LAG + 4) as epool,
            tc.tile_pool(name="opool", bufs=2) as opool,
            tc.tile_pool(name="bpool", bufs=1) as bpool,
            tc.tile_pool(name="cpool", bufs=2) as cpool,
            tc.tile_pool(name="scps", bufs=2, space="PSUM") as scps,
            tc.tile_pool(name="acps", bufs=1, space="PSUM") as acps,
            tc.tile_pool(name="ppps", bufs=2, space="PSUM") as ppps,
        ):
            # ---- persistent SBUF ------------------------------------------
            qht = [persist.tile([128, S], bf16, tag=f"qht{p}", name=f"qht{p}") for p in range(NP)]
            kht = [persist.tile([128, S], bf16, tag=f"kht{p}", name=f"kht{p}") for p in range(NP)]
            vh = [persist.tile([128, NT * 2 * VW], bf16, tag=f"vh{p}", name=f"vh{p}") for p in range(NP)]
            ot = [persist.tile([128, S], bf16, tag=f"ot{p}", name=f"ot{p}") for p in range(NP)]
            eye = persist.tile([128, 128], bf16, tag="eye")
            wo_sb = persist.tile([128, NP * DOUT], bf16, tag="wo")
            NKF = KA // 128           # full 128-row contraction tiles
            wtq_all = persist.tile([128, NK * DC], bf16, tag="wtq", name="wtq_all")
            wtk_all = persist.tile([128, NK * DC], bf16, tag="wtk", name="wtk_all")
            wtv_all = persist.tile([128, NK * DC], bf16, tag="wtv", name="wtv_all")
            wtq = [wtq_all[:, k * DC:(k + 1) * DC] for k in range(NK)]
            wtk = [wtk_all[:, k * DC:(k + 1) * DC] for k in range(NK)]
            wtv = [wtv_all[:, k * DC:(k + 1) * DC] for k in range(NK)]
            xkt = [persist.tile([128, S], bf16, tag=f"xkt{k}", name=f"xkt{k}") for k in range(NK)]

            def dma_w_all(dst_all, wdram):
                # 4 parallel-queue DMAs for the [KA, DC] weight block
                step = max(2, NKF // 4)
                for k0 in range(0, NKF, step):
                    k1 = min(k0 + step, NKF)
                    nc.sync.dma_start(
                        dst_all[:, k0 * DC: k1 * DC].rearrange("p (k c) -> p k c", c=DC),
                        wdram[k0 * 128: k1 * 128, :].rearrange("(k p) c -> p k c", p=128),
                    )
                if KA > NKF * 128:  # aug bias row
                    nc.sync.dma_start(
                        dst_all[0:1, NKF * DC:(NKF + 1) * DC], wdram[NKF * 128:, :]
                    )

            # ---- helpers ---------------------------------------------------
            def dma_x_chunk(pool, xdram, tag, n):
                big = pool.tile([128, NK * 512], bf16, tag=tag, name=tag)
                for k0 in range(0, NKF, 2):
                    nc.sync.dma_start(
                        big[:, k0 * 512:(k0 + 2) * 512].rearrange(
                            "p (k c) -> p k c", c=512
                        ),
                        xdram[k0 * 128:(k0 + 2) * 128, n * 512:(n + 1) * 512]
                        .rearrange("(k p) c -> p k c", p=128),
                    )
                if KA > NKF * 128:
                    nc.sync.dma_start(
                        big[0:1, NKF * 512:(NKF + 1) * 512],
                        xdram[NKF * 128:, n * 512:(n + 1) * 512],
                    )
                return [big[:, k * 512:(k + 1) * 512] for k in range(NK)]

            def chain_q(p, n, xt):
                ps = ppps.tile([128, 512], fp32, tag="pp")
                for k in range(NK):
                    nc.tensor.matmul(
                        ps[:], wtq[k][: ksz(k), p * 128:(p + 1) * 128],
                        xt[k][: ksz(k), :],
                        start=(k == 0), stop=(k == NK - 1),
                    )
                nc.vector.tensor_copy(qht[p][:, n * 512:(n + 1) * 512], ps[:])

            def chain_k(p, c):
                ps = ppps.tile([128, 512], fp32, tag="pp")
                for k in range(NK):
                    nc.tensor.matmul(
                        ps[:], wtk[k][: ksz(k), p * 128:(p + 1) * 128],
                        xkt[k][: ksz(k), c * 512:(c + 1) * 512],
                        start=(k == 0), stop=(k == NK - 1),
                    )
                nc.vector.tensor_copy(kht[p][:, c * 512:(c + 1) * 512], ps[:])

            def chain_v(t, xt, p0, p1):
                """V projection for t-block t, pairs [p0, p1) -> vh slots."""
                npair = p1 - p0
                col = t * 128 % 512
                ps = ppps.tile([128, 512], fp32, tag="pp")
                for k in range(NK):
                    nc.tensor.matmul(
                        ps[:, : npair * 128],
                        xt[k][: ksz(k), col: col + 128],
                        wtv[k][: ksz(k), p0 * 128: p1 * 128],
                        start=(k == 0), stop=(k == NK - 1),
                    )
                for p in range(p0, p1):
                    dst = vh[p][:, t * 2 * VW: t * 2 * VW + 2 * VW].rearrange(
                        "p (h c) -> p h c", c=VW
                    )[:, :, 0:DK]
                    src = ps[:, (p - p0) * 128: (p - p0 + 1) * 128].rearrange(
                        "p (h c) -> p h c", c=DK
                    )
                    nc.vector.tensor_copy(dst, src)

            c_osb = {}

            c_ps = {}

            def chain_c(qt, nd, half):
                """Output projection row block qt, 512-col half nd, 2 MMs."""
                if half == 0:
                    c_ps[qt] = ppps.tile([128, 512], fp32, tag="pp", name="pp")
                ps = c_ps[qt]
                for p in (2 * half, 2 * half + 1):
                    nc.tensor.matmul(
                        ps[:], ot[p][:, qt * 128:(qt + 1) * 128],
                        wo_sb[:, p * DOUT + nd * 512: p * DOUT + (nd + 1) * 512],
                        start=(p == 0), stop=(p == NP - 1),
                    )
                if half == 0:
                    return
                del c_ps[qt]
                if nd == 0:
                    c_osb[qt] = cpool.tile([128, DOUT], fp32, tag="osb", name="osb")
                osb = c_osb[qt]
                nc.vector.tensor_copy(osb[:, nd * 512:(nd + 1) * 512], ps[:])
                if nd == ND - 1:
                    del c_osb[qt]
                    nc.sync.dma_start(outp[qt * 128:(qt + 1) * 128, :], osb[:])

            def normalize(p, n, acc):
                # bf16 numerator evict + fp32 denominator DMA free the single
                # PSUM accumulator in ~1.3us
                osb = opool.tile([128, 1024], bf16, tag="nosb")
                nc.vector.tensor_copy(osb[0:65, :], acc[0:65, :])
                den = bpool.tile([128, 1024], bf16, tag="den")
                nc.sync.dma_start(den[0:1, :], osb[64:65, :])
                denf = bpool.tile([128, 1024], fp32, tag="denf")
                nc.vector.tensor_copy(denf[0:1, :], den[0:1, :])
                rd = bpool.tile([128, 1024], fp32, tag="rd")
                nc.vector.reciprocal_approx_fast(rd[0:1, :], denf[0:1, :])
                bca = bpool.tile([64, 1024], fp32, tag="bca")
                nc.gpsimd.partition_broadcast(bca[:], rd[0:1, :], channels=64)
                qof = n * 512
                nc.vector.tensor_mul(
                    ot[p][0:64, qof: qof + 512], osb[0:64, 0:512], bca[:, 0:512]
                )
                tmpb = bpool.tile([64, 512], bf16, tag="tmpb")
                nc.vector.tensor_mul(tmpb[:], osb[0:64, 512:1024], bca[:, 512:1024])
                nc.sync.dma_start(ot[p][64:128, qof: qof + 512], tmpb[:])

            # ---- background queue (credit-paced PE work) -------------------
            bg = deque()
            bg_pair = [deque() for _ in range(NP)]
            credit = [0.0]

            def run_bg(rate, cap=4500.0):
                credit[0] = min(credit[0] + rate, cap)
                while bg and (credit[0] > 0 or bg[0][0] == 0):
                    cost, thunk = bg.popleft()
                    thunk()
                    credit[0] -= cost

            def refill(p):
                src = bg_pair[p]
                while src and len(bg) < 8:
                    bg.append(src.popleft())

            def drain_all():
                for p in range(NP):
                    while bg_pair[p]:
                        bg.append(bg_pair[p].popleft())
                while bg:
                    _, thunk = bg.popleft()
                    thunk()

            store = {}

            def mk_dma(pool, xdram, tag, n, key):
                def f():
                    store[key] = dma_x_chunk(pool, xdram, tag, n)
                return (0, f)

            def mk_q(p, n):
                return (NK * 512, lambda: chain_q(p, n, store[("q", p, n)]))

            def mk_k(p, c):
                return (NK * 512, lambda: chain_k(p, c, store[("k", p, c)]))

            def mk_v(t, key, p0, p1):
                return (NK * 128 * (p1 - p0), lambda: chain_v(t, store[key], p0, p1))

            # ---- prologue --------------------------------------------------
            def dma_xk_half(h):
                for k in range(NK):
                    nc.sync.dma_start(
                        xkt[k][: ksz(k), h * 1024:(h + 1) * 1024],
                        xk[k * 128: k * 128 + ksz(k), h * 1024:(h + 1) * 1024],
                    )
            dma_w_all(wtk_all, wk)
            dma_xk_half(0)
            store[("q", 0)] = dma_x_chunk(xqp, xq, "xq", 0)
            dma_w_all(wtq_all, wq)
            dma_xk_half(1)
            dma_w_all(wtv_all, wv)
            store[("va", 0)] = dma_x_chunk(xvp, xv, "xv", 0)
            nc.sync.dma_start(eye[:], eyec[:])
            for p in range(NP):
                nc.vector.memset(vh[p][:], 1.0)
            store[("va", 1)] = dma_x_chunk(xvp, xv, "xv", 1)
            store[("q", 1)] = dma_x_chunk(xqp, xq, "xq", 1)

            chain_k(0, 0)
            chain_k(0, 1)
            chain_q(0, 0, store[("q", 0)])
            chain_k(0, 2)
            chain_k(0, 3)
            for t in range(4):
                chain_v(t, store[("va", 0)], 0, 1)

            # ---- background schedules per pair -----------------------------
            def mk_q(p, key):
                return (NK * 512, lambda: chain_q(p, key[1], store[key]))

            def mk_k(p, c):
                return (NK * 512, lambda: chain_k(p, c))

            def mk_v(t, key, p0, p1):
                return (NK * 128 * (p1 - p0), lambda: chain_v(t, store[key], p0, p1))

            # pair 0: V(p0) t4..15 + Q(0,n1..3)/Q(1,*) interleaved by deadline,
            # then K(1), then V(pairs1-3) t0..7.
            b0 = bg_pair[0]
            b0.append(mk_dma(xvp, xv, "xv", 2, ("va", 2)))
            for t in range(4, 8):
                b0.append(mk_v(t, ("va", 1), 0, 1))
            b0.append(mk_q(1, ("q", 0)))
            b0.append(mk_dma(xvp, xv, "xv", 3, ("va", 3)))
            for t in range(8, 12):
                b0.append(mk_v(t, ("va", 2), 0, 1))
            b0.append(mk_dma(xvp, xv, "xv", 0, ("vb", 0)))
            b0.append(mk_v(12, ("va", 3), 0, 1))
            b0.append(mk_v(13, ("va", 3), 0, 1))
            b0.append(mk_q(0, ("q", 1)))
            b0.append(mk_v(14, ("va", 3), 0, 1))
            b0.append(mk_v(15, ("va", 3), 0, 1))
            b0.append(mk_q(1, ("q", 1)))
            b0.append(mk_dma(xqp, xq, "xq", 2, ("q", 2)))
            b0.append(mk_q(0, ("q", 2)))
            b0.append(mk_dma(xvp, xv, "xv", 1, ("vb", 1)))
            b0.append(mk_q(1, ("q", 2)))
            b0.append(mk_dma(xqp, xq, "xq", 3, ("q", 3)))
            b0.append(mk_q(0, ("q", 3)))
            b0.append(mk_q(1, ("q", 3)))
            for c in range(4):
                b0.append(mk_k(1, c))
            for t in range(0, 4):
                b0.append(mk_v(t, ("vb", 0), 1, 4))
            b0.append(mk_dma(xvp, xv, "xv", 2, ("vb", 2)))
            for t in range(4, 8):
                b0.append(mk_v(t, ("vb", 1), 1, 4))

            # pair 1: V(pairs1-3) t8..15 (own-pair deadlines), wo, K(2),
            # then xq pass-2 first half: Q(2..3, n0..1).
            b1 = bg_pair[1]
            b1.append(mk_dma(xvp, xv, "xv", 3, ("vb", 3)))
            for t in range(8, 12):
                b1.append(mk_v(t, ("vb", 2), 1, 4))
            for t in range(12, 16):
                b1.append(mk_v(t, ("vb", 3), 1, 4))

            def dma_wo():
                nc.sync.dma_start(
                    wo_sb[:].rearrange("p (g c) -> p g c", c=DOUT),
                    wo[:].rearrange("(g p) c -> p g c", p=128),
                )
            b1.append((0, dma_wo))
            for c in range(4):
                b1.append(mk_k(2, c))
            b1.append(mk_dma(xqp, xq, "xq", 0, ("q2", 0)))
            b1.append(mk_dma(xqp, xq, "xq", 1, ("q2", 1)))
            b1.append(mk_q(2, ("q2", 0)))
            b1.append(mk_q(3, ("q2", 0)))
            b1.append(mk_q(2, ("q2", 1)))
            b1.append(mk_q(3, ("q2", 1)))

            # pair 2: xq pass-2 second half + K(3)
            b2 = bg_pair[2]
            b2.append(mk_dma(xqp, xq, "xq", 2, ("q2", 2)))
            b2.append(mk_dma(xqp, xq, "xq", 3, ("q2", 3)))
            b2.append(mk_q(2, ("q2", 2)))
            b2.append(mk_q(3, ("q2", 2)))
            b2.append(mk_q(2, ("q2", 3)))
            b2.append(mk_q(3, ("q2", 3)))
            for c in range(4):
                b2.append(mk_k(3, c))

            # pair 3: output-projection chains are queued by normalize()

            # ---- main attention stream -------------------------------------
            iters = [(p, n, t) for p in range(NP) for n in range(NQ) for t in range(NT)]
            NIT = len(iters)
            e_buf = {}
            acc_buf = {}

            def emit_sc_exp(i):
                p, n, t = iters[i]
                sc = scps.tile([128, 1024], fp32, tag="sc")
                # row-tiled score matmuls: head A on PE tile (0,0), B on (64,0)
                nc.tensor.matmul(
                    sc[:, 0:512],
                    kht[p][0:64, t * 128:(t + 1) * 128],
                    qht[p][0:64, n * 512:(n + 1) * 512],
                    start=True, stop=True,
                )
                nc.tensor.matmul(
                    sc[:, 512:1024],
                    kht[p][64:128, t * 128:(t + 1) * 128],
                    qht[p][64:128, n * 512:(n + 1) * 512],
                    start=True, stop=True,
                )
                e = epool.tile([128, 1024], bf16, tag="e")
                nc.scalar.activation(
                    e[:], sc[:], mybir.ActivationFunctionType.Exp, scale=scale
                )
                off = t * 128 - n * 512
                if 0 <= off < 512:
                    nc.vector.tensor_mul(e[:, off: off + 128], e[:, off: off + 128], eye[:])
                    nc.vector.tensor_mul(
                        e[:, 512 + off: 512 + off + 128],
                        e[:, 512 + off: 512 + off + 128], eye[:],
                    )
                e_buf[i] = e

            def emit_pv(j):
                p, n, t = iters[j]
                if t == 0:
                    acc_buf[(p, n)] = acps.tile([128, 1024], fp32, tag="acc", name="acc")
                acc = acc_buf[(p, n)]
                e = e_buf.pop(j)
                vbase = t * 2 * VW
                nc.tensor.matmul(
                    acc[0:65, 0:512],
                    vh[p][:, vbase: vbase + 65],
                    e[:, 0:512],
                    start=(t == 0), stop=(t == NT - 1),
                )
                nc.tensor.matmul(
                    acc[0:65, 512:1024],
                    vh[p][:, vbase + VW: vbase + VW + 65],
                    e[:, 512:1024],
                    start=(t == 0), stop=(t == NT - 1),
                )
                if t == NT - 1:
                    normalize(p, n, acc_buf.pop((p, n)))
                    if p == NP - 1:
                        for qt in range(n * 4, n * 4 + 4):
                            for nd in range(ND):
                                for hf in range(2):
                                    bg_pair[3].append(
                                        (NP * 256,
                                         (lambda qt_=qt, nd_=nd, h_=hf: chain_c(qt_, nd_, h_)))
                                    )

            # 2-iteration batches: [sc(i), sc(i+1)] then [pv(i-4), pv(i-3)]
            # so the PE switches tiling mode once per iteration on average.
            for i in range(0, NIT, 2):
                p, n, t = iters[i]
                emit_sc_exp(i)
                emit_sc_exp(i + 1)
                if i >= LAG + 1:
                    emit_pv(i - LAG - 1)
                    emit_pv(i - LAG)
                refill(p)
                run_bg(2 * (1400.0 if p != 2 else 1150.0), cap=7000.0)
            for j in range(NIT - LAG - 1, NIT):
                emit_pv(j)
                run_bg(1400.0, cap=7000.0)
            drain_all()

    nc.compile()
    return nc


def _bf16(a):
    return np.ascontiguousarray(a).astype(ml_dtypes.bfloat16)


def _prep_core_inputs(q, k, v, Wq, bq, Wk, bk, Wv, bv, Wo, aug_bias):
    """Per-core host-side slicing/transposition. Returns list of 8 dicts."""
    eyec = _bf16(1.0 - np.eye(128, dtype=np.float32))
    maps = []
    for c in range(N_CORES):
        b = c // 2
        h0 = (c % 2) * HEADS_PER_CORE
        r0, r1 = h0 * DK, (h0 + HEADS_PER_CORE) * DK
        m = {}
        for name, x in (("xq", q[b]), ("xk", k[b]), ("xv", v[b])):
            xt = x.T  # [D, S]
            if aug_bias:
                xt = np.concatenate([xt, np.ones((1, S), np.float32)], axis=0)
            m[name] = _bf16(xt)
        for name, W, bias in (("wq", Wq, bq), ("wk", Wk, bk), ("wv", Wv, bv)):
            wt = W[r0:r1, :].T  # [D, DC]
            if aug_bias:
                wt = np.concatenate([wt, bias[None, r0:r1]], axis=0)
            m[name] = _bf16(wt)
        m["wo"] = _bf16(Wo[:, r0:r1].T)  # [DC, D]
        m["eyec"] = eyec
        maps.append(m)
    return maps


_PROGRAM_CACHE = {}


def _get_program(aug_bias):
    if aug_bias not in _PROGRAM_CACHE:
        _PROGRAM_CACHE[aug_bias] = build_attention_core(
            S=S, DIN=D, NH=HEADS_PER_CORE, DOUT=D, aug_bias=aug_bias
        )
    return _PROGRAM_CACHE[aug_bias]


def _reference_fallback(q, k, v, Wq, bq, Wk, bk, Wv, bv, Wo, bo, mask):
    """Pure-numpy fallback for unexpected mask patterns."""
    out = np.empty((B, S, D), np.float32)
    msk = np.broadcast_to(mask.reshape(mask.shape[-2], mask.shape[-1]), (S, S))
    for b in range(B):
        qh = (q[b] @ Wq.T + bq).reshape(S, H, DK).transpose(1, 0, 2)
        kh = (k[b] @ Wk.T + bk).reshape(S, H, DK).transpose(1, 0, 2)
        vh = (v[b] @ Wv.T + bv).reshape(S, H, DK).transpose(1, 0, 2)
        acc = np.empty((H, S, DK), np.float32)
        for h in range(H):
            s = (qh[h] @ kh[h].T) / np.float32(np.sqrt(DK))
            s = np.where(msk == 0, np.finfo(np.float32).min, s)
            s = s - s.max(axis=-1, keepdims=True)
            e = np.exp(s)
            p = e / e.sum(axis=-1, keepdims=True)
            acc[h] = p @ vh[h]
        o = acc.transpose(1, 0, 2).reshape(S, D)
        out[b] = o @ Wo.T + bo
    return out


def kernel(q, k, v, Wq, bq, Wk, bk, Wv, bv, Wo, bo, mask, _trace=False):
    from concourse.bass_utils import run_bass_kernel_spmd

    q = np.asarray(q, np.float32)
    k = np.asarray(k, np.float32)
    v = np.asarray(v, np.float32)
    Wq, bq = np.asarray(Wq, np.float32), np.asarray(bq, np.float32)
    Wk, bk = np.asarray(Wk, np.float32), np.asarray(bk, np.float32)
    Wv, bv = np.asarray(Wv, np.float32), np.asarray(bv, np.float32)
    Wo, bo = np.asarray(Wo, np.float32), np.asarray(bo, np.float32)
    mask = np.asarray(mask)

    expected_mask = 1 - np.eye(S, dtype=np.int32)
    if not np.array_equal(mask.reshape(-1, S, S)[0].astype(np.int32), expected_mask):
        return _reference_fallback(q, k, v, Wq, bq, Wk, bk, Wv, bv, Wo, bo, mask)

    aug_bias = bool(np.any(bq) or np.any(bk) or np.any(bv))
    nc = _get_program(aug_bias)
    in_maps = _prep_core_inputs(q, k, v, Wq, bq, Wk, bk, Wv, bv, Wo, aug_bias)
    res = run_bass_kernel_spmd(
        nc, in_maps, core_ids=list(range(N_CORES)), trace=_trace
    )
    out = np.empty((B, S, D), np.float32)
    for b in range(B):
        out[b] = res.results[2 * b]["outp"] + res.results[2 * b + 1]["outp"] + bo
    if _trace:
        kernel.last_results = res
    return out
